# revision 9
# baseline (speedup 1.0000x reference)
"""BM3D-deblur (regularized-inverse + global empirical Wiener) on 8 Trainium2 cores.

Math (per 1024x1024 image-channel, 24 total, 3 per core):
  G = fft2(y); Z = G*ri; S = max(|Z|^2/n - psd, 0); Wf = S/(S+psd+eps)
  out = real(ifft2(Z*Wf))
with ri, psd derived from the 25x25 PSF on the host (tiny).

2D FFT via digit decomposition h = 8a+j, w = 8b+m, k_h = (2q+t)+128*kj,
k_w = kb+128*km. Pipeline (planes A/B alternate, all [128,8192] bf16):
  stageA: fused S1+T1 - y-blocks as stationary (lhsT), W1_j moving
          -> B[p=b, f=(m,j,k1)]  (contracts a, transposes b to partitions)
  S2:     per m contract b with F128*tw_m        -> A[p=kb, f=interleave]
  T2:     PE transpose                           -> B[p=(m,j,t), f=(q,kb)]
  S3:     contract (j,m) with D8xD8              -> zr/zi chunks
  Wiener: elementwise (chunked, 3-engine balanced)
  S3'+T2' fused: zw-blocks as stationary, conj(W3) moving
          -> A[p=kb, f=(q,g2)]   (contracts spec, transposes kb up)
  S2':    per m contract kb with conj(F128)*exp(2pi i m kb/N) (twiddle
          folded into 8 per-m matrices)          -> B[p=b, f=(q,t,j,m)]
  T1':    PE transpose                           -> A[p=k1, f=(j,m,b)]
  S1':    per j contract k1 (real out)           -> out[p=a, f=(j,8b+m)]
All spectral coefficient planes permuted on host into device layout.
Elementwise work (PSUM evacs + Wiener) is spread across DVE/ACT/Pool by a
static greedy balancer; ACT uses the reciprocal_and_small table (Square,
Reciprocal, Copy - no table reloads).
"""
import sys

sys.path.insert(0, "/opt/trn_rl_repo")

import numpy as np
import ml_dtypes

import concourse.bass as bass
import concourse.bacc as bacc
import concourse.tile as tile
from concourse import mybir
from concourse.bass_utils import run_bass_kernel_spmd
import concourse.hw_specs as _hw_specs

_orig_get_tables = _hw_specs.get_activation_tables


def _patched_tables(arch):
    t = dict(_orig_get_tables(arch))
    pin = {
        mybir.ActivationFunctionType.Ln,
        mybir.ActivationFunctionType.Exp,
        mybir.ActivationFunctionType.Square,
    }
    for name in list(t):
        if name != "natural_log_exp_and_others" and (t[name] & pin):
            t[name] = t[name] - pin
    return t


bacc.get_activation_tables = _patched_tables

N = 1024
SIGMA = 0.05
CSUB = float(SIGMA**2 * N * N * N * N)  # psd = sigma^2 * n^2 * a
N_CORES = 8
IMGS = 3  # images per core
CH = 1024  # wiener chunk cols

BDT = mybir.dt.bfloat16
F32 = mybir.dt.float32
BF = ml_dtypes.bfloat16
AF = mybir.ActivationFunctionType
ALU = mybir.AluOpType


# ---------------------------------------------------------------- host math
def _host_consts(psf25: np.ndarray) -> dict[str, np.ndarray]:
    ar = np.arange(128)
    F128 = np.exp(-2j * np.pi * np.outer(ar, ar) / 128)
    D8 = np.exp(-2j * np.pi * np.outer(np.arange(8), np.arange(8)) / 8)
    tw = np.exp(-2j * np.pi * np.outer(np.arange(8), ar) / N)  # [j, k1]

    # forward W1 per j (moving operand of fused stage-A): [a, (j, c2, k1)]
    w1f = np.empty((128, 8, 2, 128), np.float32)
    for j in range(8):
        Wj = F128 * tw[j][None, :]
        w1f[:, j, 0] = Wj.real
        w1f[:, j, 1] = Wj.imag
    # w2f: same matrices, planes (re, im, -im)
    w2f = np.empty((128, 8, 3, 128), np.float32)
    for m in range(8):
        Wm = F128 * tw[m][None, :]
        w2f[:, m, 0] = Wm.real
        w2f[:, m, 1] = Wm.imag
        w2f[:, m, 2] = -Wm.imag
    # W3 fwd: rows g_in=16m+2j+t (T2 col enum), cols spec p=16kj+2km+t
    # W3i inv: rows spec p, cols g2=64t+8j+m
    W3 = np.zeros((128, 128), complex)
    W3i = np.zeros((128, 128), complex)
    for j in range(8):
        for m in range(8):
            for kj in range(8):
                for km in range(8):
                    v = D8[j, kj] * D8[m, km]
                    for t in range(2):
                        W3[16 * m + 2 * j + t, 16 * kj + 2 * km + t] = v
                        W3i[16 * kj + 2 * km + t, 64 * t + 8 * j + m] = np.conj(v)
    w3f = np.stack([W3.real, W3.imag, -W3.imag], 1).astype(np.float32)  # [128,3,128]
    w3i = np.stack([W3i.real, W3i.imag, -W3i.imag], 1).astype(np.float32)
    # inverse S2' lhsT per m: [kb, b] = conj(F128) * exp(+2pi i m kb / N)
    # (inverse W-axis twiddle folded in); planes (re, im, -im)
    wfim = np.empty((128, 8, 3, 128), np.float32)
    kb = np.arange(128)
    for m in range(8):
        Vm = np.conj(F128) * np.exp(2j * np.pi * m * kb / N)[:, None]
        wfim[:, m, 0] = Vm.real
        wfim[:, m, 1] = Vm.imag
        wfim[:, m, 2] = -Vm.imag
    # inverse S1' lhsT per j: [k1, a] = conj(W1_j).T ; planes (re, -im) (real out)
    w1i = np.empty((128, 8, 2, 128), np.float32)
    for j in range(8):
        V = np.conj(F128 * tw[j][None, :]).T
        w1i[:, j, 0] = V.real
        w1i[:, j, 1] = -V.imag
    # wiener planes in spectral device layout
    P = np.zeros((N, N))
    P[:25, :25] = psf25
    P = np.roll(P, (-12, -12), axis=(0, 1))
    Hf = np.fft.fft2(P)
    ri = np.conj(Hf) / (np.abs(Hf) ** 2 + SIGMA**2)
    p = np.arange(128)
    kj, km, t = p // 16, (p % 16) // 2, p % 2
    f = np.arange(8192)
    q, kbf = f // 128, f % 128
    kh = (2 * q[None, :] + t[:, None]) + 128 * kj[:, None]
    kw = kbf[None, :] + 128 * km[:, None]
    rr_dev = (ri.real / (N * N))[kh, kw]
    rii_dev = (ri.imag / (N * N))[kh, kw]
    wien = np.stack([rr_dev, rii_dev], 1).astype(np.float32)  # [128,2,8192]

    bf = lambda x: np.ascontiguousarray(x.astype(BF))
    return {
        "w1f": bf(w1f.reshape(128, 8 * 2 * 128)),
        "w2f": bf(w2f.reshape(128, 8 * 3 * 128)),
        "w3f": bf(w3f.reshape(128, 3 * 128)),
        "w3i": bf(w3i.reshape(128, 3 * 128)),
        "wfim": bf(wfim.reshape(128, 8 * 3 * 128)),
        "w1i": bf(w1i.reshape(128, 8 * 2 * 128)),
        "wien": bf(wien.reshape(128, 2 * 8192)),
        "ident": bf(np.eye(128, dtype=np.float32)),
    }


# ---------------------------------------------------------------- balancer
class EW:
    """Static greedy balancer for elementwise work across DVE/ACT/Pool."""

    def __init__(self, nc, pool_psum=False):
        self.nc = nc
        self.pool_psum = pool_psum
        self.load = {"v": 0.0, "a": 0.0, "p": 0.0}

    def _cost(self, e, free, two_byte, psum):
        if e == "v":
            return free * 1.04 * (0.5 if two_byte else 1.0) + (130 if psum else 62) + 70
        if e == "a":
            return free * 0.833 + 170 + 57
        return free * 0.833 / 0.5 + 131

    def _pick(self, allowed, free, two_byte, psum):
        e = min(allowed, key=lambda x: self.load[x] + self._cost(x, free, two_byte, psum))
        self.load[e] += self._cost(e, free, two_byte, psum)
        return e

    def evac(self, dst, src):
        free = int(np.prod(src.shape[1:]))
        two = mybir.dt.size(src.dtype) == 2 and mybir.dt.size(dst.dtype) == 2
        allowed = ["v", "a"] + (["p"] if self.pool_psum else [])
        e = self._pick(allowed, free, two, True)
        if e == "v":
            self.nc.vector.tensor_copy(dst, src)
        elif e == "a":
            self.nc.scalar.copy(dst, src)
        else:
            self.nc.gpsimd.tensor_copy(dst, src)

    def tt(self, op, dst, a, b):
        free = int(np.prod(dst.shape[1:]))
        e = self._pick(["v", "p"], free, True, False)
        eng = self.nc.vector if e == "v" else self.nc.gpsimd
        eng.tensor_tensor(dst, a, b, op)

    def sq(self, dst, src):
        # square: ACT Square or mul on DVE/Pool
        free = int(np.prod(dst.shape[1:]))
        e = self._pick(["v", "a", "p"], free, True, False)
        if e == "a":
            self.nc.scalar.activation(dst, src, AF.Square)
        else:
            eng = self.nc.vector if e == "v" else self.nc.gpsimd
            eng.tensor_tensor(dst, src, src, ALU.mult)

    def ts(self, dst, src, s0, s1, op0, op1):
        free = int(np.prod(dst.shape[1:]))
        e = self._pick(["v", "p"], free, True, False)
        eng = self.nc.vector if e == "v" else self.nc.gpsimd
        eng.tensor_scalar(dst, src, s0, s1, op0=op0, op1=op1)

    def maxs(self, dst, src, s0):
        free = int(np.prod(dst.shape[1:]))
        e = self._pick(["v", "p"], free, True, False)
        eng = self.nc.vector if e == "v" else self.nc.gpsimd
        eng.tensor_scalar_max(dst, src, s0)

    def act(self, dst, src, func, scale=1.0):
        free = int(np.prod(dst.shape[1:]))
        self.load["a"] += self._cost("a", free, True, False)
        self.nc.scalar.activation(dst, src, func, scale=scale)


# ---------------------------------------------------------------- device IR
def build_program(n_imgs: int = IMGS, dbg_stage: str | None = None):
    nc = bacc.Bacc("TRN2", target_bir_lowering=False, debug=False)
    y3 = nc.dram_tensor("y3", [n_imgs, N, N], F32, kind="ExternalInput")
    o3 = nc.dram_tensor("o3", [n_imgs, N, N], F32, kind="ExternalOutput")
    if dbg_stage:
        dbr = nc.dram_tensor("dbgr", [128, 8192], F32, kind="ExternalOutput")
        dbi = nc.dram_tensor("dbgi", [128, 8192], F32, kind="ExternalOutput")
    dw = {
        k: nc.dram_tensor(k, list(s), BDT, kind="ExternalInput")
        for k, s in {
            "w1f": (128, 2048),
            "w2f": (128, 3072),
            "w3f": (128, 384),
            "w3i": (128, 384),
            "wfim": (128, 3072),
            "w1i": (128, 2048),
            "wien": (128, 16384),
            "ident": (128, 128),
        }.items()
    }

    with tile.TileContext(nc) as tc:
        import contextlib

        with contextlib.ExitStack() as ctx:
            const = ctx.enter_context(tc.tile_pool(name="const", bufs=1))
            plan = ctx.enter_context(tc.tile_pool(name="plan", bufs=1))
            ypool = ctx.enter_context(tc.tile_pool(name="ypool", bufs=2))
            ps = ctx.enter_context(tc.tile_pool(name="ps", bufs=4, space="PSUM"))
            ps2 = ctx.enter_context(tc.tile_pool(name="ps2", bufs=2, space="PSUM"))
            tmp = ctx.enter_context(tc.tile_pool(name="tmp", bufs=10))
            zw = ctx.enter_context(tc.tile_pool(name="zw", bufs=3))

            ew = EW(nc, pool_psum=False)

            # constants
            sw = {}
            for k in dw:
                shp = [128, dw[k].shape[1]]
                t_ = const.tile(shp, BDT, name=k)
                nc.sync.dma_start(out=t_, in_=dw[k].ap())
                sw[k] = t_
            w1fv = sw["w1f"].rearrange("p (j ck) -> p j ck", j=8)  # ck = (c2,k1)
            w2f = sw["w2f"].rearrange("p (m c k) -> p m c k", m=8, c=3)
            w3f = sw["w3f"].rearrange("p (c k) -> p c k", c=3)
            w3iv = sw["w3i"].rearrange("p (c k) -> p c k", c=3)
            wfim = sw["wfim"].rearrange("p (m c k) -> p m c k", m=8, c=3)
            w1i = sw["w1i"].rearrange("p (j c k) -> p j c k", j=8, c=2)
            wien = sw["wien"].rearrange("p (c f) -> p c f", c=2)
            ident = sw["ident"]

            def _snap(stage, br, bi):
                if dbg_stage == stage:
                    nc.gpsimd.dma_start(out=dbr.ap(), in_=br)
                    nc.gpsimd.dma_start(out=dbi.ap(), in_=bi)

            # persistent plan buffers (bf16 [128, 8192] each)
            Ar = plan.tile([128, 8192], BDT, name="Ar")
            Ai = plan.tile([128, 8192], BDT, name="Ai")
            Br = plan.tile([128, 8192], BDT, name="Br")
            Bi = plan.tile([128, 8192], BDT, name="Bi")

            for img in range(n_imgs):
                # ---- load (fp32 HBM -> bf16 SBUF, SWDGE cast)
                y_t = ypool.tile([128, 8192], BDT)
                nc.gpsimd.dma_start(
                    out=y_t, in_=y3.ap()[img].rearrange("(p j) w -> p (j w)", j=8)
                )

                # ---- stage A (fused S1+T1): y blocks stationary, W1_j moving
                # out B[p=b, f = 1024m + 128j + k1] (complex)
                yv = y_t.rearrange("p (j b m) -> p j m b", j=8, b=128)
                for m in range(8):
                    for g in range(2):  # j groups of 4 -> one [128,1024] psum tile
                        pt = ps2.tile([128, 1024], F32, tag="p2")
                        for jj in range(4):
                            j = 4 * g + jj
                            nc.tensor.matmul(
                                pt[:, 256 * jj : 256 * (jj + 1)],
                                yv[:, j, m],
                                w1fv[:, j],
                                start=True,
                                stop=True,
                            )
                        ptv = pt.rearrange("p (jj c k) -> p jj c k", jj=4, c=2)
                        sl = slice(1024 * m + 512 * g, 1024 * m + 512 * (g + 1))
                        dr = Br[:, sl].rearrange("p (jj k) -> p jj k", jj=4)
                        di = Bi[:, sl].rearrange("p (jj k) -> p jj k", jj=4)
                        ew.evac(dr, ptv[:, :, 0, :])
                        ew.evac(di, ptv[:, :, 1, :])

                _snap("A", Br, Bi)
                # ---- S2: per m contract b -> A[p=kb, f=interleaved]
                for m in range(8):
                    for c in range(2):
                        jr = Br[:, 1024 * m + 512 * c : 1024 * m + 512 * (c + 1)]
                        ji = Bi[:, 1024 * m + 512 * c : 1024 * m + 512 * (c + 1)]
                        pr = ps.tile([128, 512], F32, tag="pp")
                        pi = ps.tile([128, 512], F32, tag="pp")
                        nc.tensor.matmul(pr, w2f[:, m, 0], jr, start=True, stop=False)
                        nc.tensor.matmul(pi, w2f[:, m, 1], jr, start=True, stop=False)
                        nc.tensor.matmul(pr, w2f[:, m, 2], ji, start=False, stop=True)
                        nc.tensor.matmul(pi, w2f[:, m, 0], ji, start=False, stop=True)
                        # interleaved evac: psum (jj,q,t) -> f = 128q + 16m + 8c + 2jj + t
                        for dstp, srcp in ((Ar, pr), (Ai, pi)):
                            sview = srcp.rearrange("p (jj q t) -> p jj q t", jj=4, q=64)
                            dview = dstp.rearrange(
                                "p (q mm cc jj t) -> p q mm cc jj t",
                                q=64, mm=8, cc=2, jj=4,
                            )[:, :, m, c, :, :].transpose([0, 2, 1, 3])
                            ew.evac(dview, sview)

                _snap("S2", Ar, Ai)
                # ---- T2: A -> B[p=g_in=(m,j,t), f=(q,kb)]
                for g in range(8):  # groups of 8 q
                    for src, dst in ((Ar, Br), (Ai, Bi)):
                        pt = ps.tile([128, 1024], BDT, tag="pp")
                        for qq in range(8):
                            q = 8 * g + qq
                            nc.tensor.transpose(
                                pt[:, 128 * qq : 128 * (qq + 1)],
                                src[:, 128 * q : 128 * (q + 1)],
                                ident,
                            )
                        ew.evac(dst[:, 1024 * g : 1024 * (g + 1)], pt)

                _snap("T2", Br, Bi)
                # ---- S3 + Wiener + fused S3'+T2' : B -> A[p=kb, f=(q,g2)]
                nch = 8192 // CH
                for c in range(nch):
                    sl = slice(CH * c, CH * (c + 1))
                    zr = zw.tile([128, CH], BDT, tag="zr")
                    zi = zw.tile([128, CH], BDT, tag="zi")
                    for hh in range(CH // 512):
                        hsl = slice(512 * hh, 512 * (hh + 1))
                        bsl = slice(CH * c + 512 * hh, CH * c + 512 * (hh + 1))
                        pr = ps.tile([128, 512], F32, tag="pp")
                        pi = ps.tile([128, 512], F32, tag="pp")
                        nc.tensor.matmul(pr, w3f[:, 0], Br[:, bsl], start=True, stop=False)
                        nc.tensor.matmul(pi, w3f[:, 1], Br[:, bsl], start=True, stop=False)
                        nc.tensor.matmul(pr, w3f[:, 2], Bi[:, bsl], start=False, stop=True)
                        nc.tensor.matmul(pi, w3f[:, 0], Bi[:, bsl], start=False, stop=True)
                        ew.evac(zr[:, hsl], pr)
                        ew.evac(zi[:, hsl], pi)
                    rrc = wien[:, 0, sl]
                    ric = wien[:, 1, sl]
                    t1 = tmp.tile([128, CH], BDT, tag="wt")
                    ew.sq(t1, zr)
                    t2 = tmp.tile([128, CH], BDT, tag="wt")
                    ew.sq(t2, zi)
                    mm_ = tmp.tile([128, CH], BDT, tag="wt")
                    ew.tt(ALU.add, mm_, t1, t2)
                    rc = tmp.tile([128, CH], BDT, tag="wt")
                    ew.ts(rc, mm_, CSUB, 0.0, ALU.subtract, ALU.max)
                    u2 = tmp.tile([128, CH], F32, tag="wtf", bufs=2)
                    ew.maxs(u2, mm_, CSUB)
                    ln_ = tmp.tile([128, CH], F32, tag="wtf", bufs=2)
                    ew.act(ln_, u2, AF.Ln)
                    r_ = tmp.tile([128, CH], BDT, tag="wt")
                    ew.act(r_, ln_, AF.Exp, scale=-1.0)
                    w_ = tmp.tile([128, CH], BDT, tag="wt")
                    ew.tt(ALU.mult, w_, rc, r_)
                    fr = tmp.tile([128, CH], BDT, tag="wt")
                    ew.tt(ALU.mult, fr, w_, rrc)
                    fi = tmp.tile([128, CH], BDT, tag="wt")
                    ew.tt(ALU.mult, fi, w_, ric)
                    p1 = tmp.tile([128, CH], BDT, tag="wt")
                    ew.tt(ALU.mult, p1, zr, fr)
                    p2 = tmp.tile([128, CH], BDT, tag="wt")
                    ew.tt(ALU.mult, p2, zi, fi)
                    zwr = zw.tile([128, CH], BDT, tag="zwr")
                    ew.tt(ALU.subtract, zwr, p1, p2)
                    p3 = tmp.tile([128, CH], BDT, tag="wt")
                    ew.tt(ALU.mult, p3, zr, fi)
                    p4 = tmp.tile([128, CH], BDT, tag="wt")
                    ew.tt(ALU.mult, p4, zi, fr)
                    zwi = zw.tile([128, CH], BDT, tag="zwi")
                    ew.tt(ALU.add, zwi, p3, p4)
                    # fused S3'+T2': zw blocks stationary, conj(W3) moving
                    ptr = ps2.tile([128, CH], F32, tag="p2")
                    pti = ps2.tile([128, CH], F32, tag="p2")
                    for qq in range(CH // 128):
                        qsl = slice(128 * qq, 128 * (qq + 1))
                        nc.tensor.matmul(ptr[:, qsl], zwr[:, qsl], w3iv[:, 0], start=True, stop=False)
                        nc.tensor.matmul(pti[:, qsl], zwr[:, qsl], w3iv[:, 1], start=True, stop=False)
                        nc.tensor.matmul(ptr[:, qsl], zwi[:, qsl], w3iv[:, 2], start=False, stop=True)
                        nc.tensor.matmul(pti[:, qsl], zwi[:, qsl], w3iv[:, 0], start=False, stop=True)
                    ew.evac(Ar[:, sl], ptr)
                    ew.evac(Ai[:, sl], pti)

                _snap("S3p", Ar, Ai)
                # ---- S2': per m contract kb (twiddle-folded conj(F128)) -> B[p=b, f=(q,t,j,m)]
                Avr = Ar.rearrange("p (q t j m) -> p q t j m", q=64, t=2, j=8)
                Avi = Ai.rearrange("p (q t j m) -> p q t j m", q=64, t=2, j=8)
                Bvr = Br.rearrange("p (q t j m) -> p q t j m", q=64, t=2, j=8)
                Bvi = Bi.rearrange("p (q t j m) -> p q t j m", q=64, t=2, j=8)
                for m in range(8):
                    for c in range(2):
                        qsl = slice(32 * c, 32 * (c + 1))
                        jr = Avr[:, qsl, :, :, m]
                        ji = Avi[:, qsl, :, :, m]
                        pr = ps.tile([128, 512], F32, tag="pp")
                        pi = ps.tile([128, 512], F32, tag="pp")
                        nc.tensor.matmul(pr, wfim[:, m, 0], jr, start=True, stop=False)
                        nc.tensor.matmul(pi, wfim[:, m, 1], jr, start=True, stop=False)
                        nc.tensor.matmul(pr, wfim[:, m, 2], ji, start=False, stop=True)
                        nc.tensor.matmul(pi, wfim[:, m, 0], ji, start=False, stop=True)
                        prv = pr.rearrange("p (q t j) -> p q t j", q=32, t=2)
                        piv = pi.rearrange("p (q t j) -> p q t j", q=32, t=2)
                        ew.evac(Bvr[:, qsl, :, :, m], prv)
                        ew.evac(Bvi[:, qsl, :, :, m], piv)

                _snap("S2p", Br, Bi)
                # ---- T1': B[p=b, f=(q,t,j,m)] -> A[p=k1, f=(j,m,b)]
                vB4r = Br.rearrange("p (q t j m) -> p j m (q t)", q=64, t=2, j=8)
                vB4i = Bi.rearrange("p (q t j m) -> p j m (q t)", q=64, t=2, j=8)
                for j in range(8):
                    for src, dst in ((vB4r, Ar), (vB4i, Ai)):
                        pt = ps.tile([128, 1024], BDT, tag="pp")
                        for mj in range(8):
                            nc.tensor.transpose(
                                pt[:, 128 * mj : 128 * (mj + 1)], src[:, j, mj], ident
                            )
                        ew.evac(dst[:, 1024 * j : 1024 * (j + 1)], pt)

                _snap("T1p", Ar, Ai)
                # ---- S1': per j contract k1 (real out) -> out[p=a, f=(j, 8b+m)]
                out_t = ypool.tile([128, 8192], BDT, tag="y_t")
                for j in range(8):
                    for c in range(2):
                        off = 1024 * j + 512 * c
                        jr = Ar[:, off : off + 512]
                        ji = Ai[:, off : off + 512]
                        pr = ps.tile([128, 512], F32, tag="pp")
                        nc.tensor.matmul(pr, w1i[:, j, 0], jr, start=True, stop=False)
                        nc.tensor.matmul(pr, w1i[:, j, 1], ji, start=False, stop=True)
                        # evac with digit swap (m,b) -> 8b+m
                        src = pr.rearrange("p (m b) -> p m b", m=4)
                        dst = out_t.rearrange("p (j b m) -> p j b m", j=8, b=128)[
                            :, j, :, 4 * c : 4 * (c + 1)
                        ].transpose([0, 2, 1])
                        ew.evac(dst, src)
                _snap("OUT", out_t, out_t)
                nc.gpsimd.dma_start(
                    out=o3.ap()[img].rearrange("(p j) w -> p (j w)", j=8), in_=out_t
                )

    nc.compile()
    return nc


_PROG = None


def _get_prog():
    global _PROG
    if _PROG is None:
        _PROG = build_program(IMGS)
    return _PROG


def kernel(y: np.ndarray, psf: np.ndarray) -> np.ndarray:
    consts = _host_consts(np.asarray(psf, np.float64)[0, 0])
    nc = _get_prog()
    y24 = np.ascontiguousarray(np.asarray(y, np.float32).reshape(N_CORES * IMGS, N, N))
    in_maps = []
    for c in range(N_CORES):
        m = dict(consts)
        m["y3"] = y24[IMGS * c : IMGS * (c + 1)]
        in_maps.append(m)
    res = run_bass_kernel_spmd(nc, in_maps, core_ids=list(range(N_CORES)))
    out = np.stack([res.results[c]["o3"] for c in range(N_CORES)])
    return out.reshape(8, 3, N, N).astype(np.float32)


# revision 13
# speedup vs baseline: 1.1474x; 1.1474x over previous
"""BM3D-deblur (regularized-inverse + global empirical Wiener) on 8 Trainium2 cores.

Math (per 1024x1024 image-channel, 24 total, 3 per core):
  G = fft2(y); Z = G*ri; S = max(|Z|^2/n - psd, 0); Wf = S/(S+psd+eps)
  out = real(ifft2(Z*Wf))
with ri, psd derived from the 25x25 PSF on the host (tiny).

2D FFT via digit decomposition h = 8a+j, w = 8b+m, k_h = (2q+t)+128*kj,
k_w = kb+128*km. Pipeline (planes A/B alternate, all [128,8192] bf16):
  stageA: fused S1+T1 - y-blocks as stationary (lhsT), W1_j moving
          -> B[p=b, f=(m,j,k1)]  (contracts a, transposes b to partitions)
  S2:     per m contract b with F128*tw_m        -> A[p=kb, f=interleave]
  T2:     PE transpose                           -> B[p=(m,j,t), f=(q,kb)]
  S3:     contract (j,m) with D8xD8              -> zr/zi chunks
  Wiener: elementwise (chunked, 3-engine balanced)
  S3'+T2' fused: zw-blocks as stationary, conj(W3) moving
          -> A[p=kb, f=(q,g2)]   (contracts spec, transposes kb up)
  S2':    per m contract kb with conj(F128)*exp(2pi i m kb/N) (twiddle
          folded into 8 per-m matrices)          -> B[p=b, f=(q,t,j,m)]
  T1':    PE transpose                           -> A[p=k1, f=(j,m,b)]
  S1':    per j contract k1 (real out)           -> out[p=a, f=(j,8b+m)]
All spectral coefficient planes permuted on host into device layout.
Elementwise work (PSUM evacs + Wiener) is spread across DVE/ACT/Pool by a
static greedy balancer; ACT uses the reciprocal_and_small table (Square,
Reciprocal, Copy - no table reloads).
"""
import sys

sys.path.insert(0, "/opt/trn_rl_repo")

import numpy as np
import ml_dtypes

import concourse.bass as bass
import concourse.bacc as bacc
import concourse.tile as tile
from concourse import mybir
from concourse.bass_utils import run_bass_kernel_spmd
import concourse.hw_specs as _hw_specs

_orig_get_tables = _hw_specs.get_activation_tables


def _patched_tables(arch):
    t = dict(_orig_get_tables(arch))
    pin = {
        mybir.ActivationFunctionType.Ln,
        mybir.ActivationFunctionType.Exp,
        mybir.ActivationFunctionType.Square,
    }
    for name in list(t):
        if name != "natural_log_exp_and_others" and (t[name] & pin):
            t[name] = t[name] - pin
    return t


bacc.get_activation_tables = _patched_tables

N = 1024
SIGMA = 0.05
CSUB = float(SIGMA**2 * N * N * N * N)  # psd = sigma^2 * n^2 * a
N_CORES = 8
IMGS = 3  # images per core
CH = 1024  # wiener chunk cols

BDT = mybir.dt.bfloat16
F32 = mybir.dt.float32
BF = ml_dtypes.bfloat16
AF = mybir.ActivationFunctionType
ALU = mybir.AluOpType


# ---------------------------------------------------------------- host math
def _host_consts(psf25: np.ndarray) -> dict[str, np.ndarray]:
    ar = np.arange(128)
    F128 = np.exp(-2j * np.pi * np.outer(ar, ar) / 128)
    D8 = np.exp(-2j * np.pi * np.outer(np.arange(8), np.arange(8)) / 8)
    tw = np.exp(-2j * np.pi * np.outer(np.arange(8), ar) / N)  # [j, k1]

    # forward W1 per j (moving operand of fused stage-A): [a, (j, c2, k1)]
    w1f = np.empty((128, 8, 2, 128), np.float32)
    for j in range(8):
        Wj = F128 * tw[j][None, :]
        w1f[:, j, 0] = Wj.real
        w1f[:, j, 1] = Wj.imag
    # w2f: same matrices, planes (re, im, -im)
    w2f = np.empty((128, 8, 3, 128), np.float32)
    for m in range(8):
        Wm = F128 * tw[m][None, :]
        w2f[:, m, 0] = Wm.real
        w2f[:, m, 1] = Wm.imag
        w2f[:, m, 2] = -Wm.imag
    # W3 fwd: rows g_in=16m+2j+t (T2 col enum), cols spec p=16kj+2km+t
    # W3i inv: rows spec p, cols g2=64t+8j+m
    W3 = np.zeros((128, 128), complex)
    W3i = np.zeros((128, 128), complex)
    for j in range(8):
        for m in range(8):
            for kj in range(8):
                for km in range(8):
                    v = D8[j, kj] * D8[m, km]
                    for t in range(2):
                        W3[16 * m + 2 * j + t, 16 * kj + 2 * km + t] = v
                        W3i[16 * kj + 2 * km + t, 64 * t + 8 * j + m] = np.conj(v)
    w3f = np.stack([W3.real, W3.imag, -W3.imag], 1).astype(np.float32)  # [128,3,128]
    w3i = np.stack([W3i.real, W3i.imag, -W3i.imag], 1).astype(np.float32)
    # inverse S2' lhsT per m: [kb, b] = conj(F128) * exp(+2pi i m kb / N)
    # (inverse W-axis twiddle folded in); planes (re, im, -im)
    wfim = np.empty((128, 8, 3, 128), np.float32)
    kb = np.arange(128)
    for m in range(8):
        Vm = np.conj(F128) * np.exp(2j * np.pi * m * kb / N)[:, None]
        wfim[:, m, 0] = Vm.real
        wfim[:, m, 1] = Vm.imag
        wfim[:, m, 2] = -Vm.imag
    # inverse S1' lhsT per j: [k1, a] = conj(W1_j).T ; planes (re, -im) (real out)
    w1i = np.empty((128, 8, 2, 128), np.float32)
    for j in range(8):
        V = np.conj(F128 * tw[j][None, :]).T
        w1i[:, j, 0] = V.real
        w1i[:, j, 1] = -V.imag
    # wiener planes in spectral device layout
    P = np.zeros((N, N))
    P[:25, :25] = psf25
    P = np.roll(P, (-12, -12), axis=(0, 1))
    Hf = np.fft.fft2(P)
    ri = np.conj(Hf) / (np.abs(Hf) ** 2 + SIGMA**2)
    p = np.arange(128)
    kj, km, t = p // 16, (p % 16) // 2, p % 2
    f = np.arange(8192)
    q, kbf = f // 128, f % 128
    kh = (2 * q[None, :] + t[:, None]) + 128 * kj[:, None]
    kw = kbf[None, :] + 128 * km[:, None]
    rr_dev = (ri.real / (N * N))[kh, kw]
    rii_dev = (ri.imag / (N * N))[kh, kw]
    wien = np.stack([rr_dev, rii_dev], 1).astype(np.float32)  # [128,2,8192]

    bf = lambda x: np.ascontiguousarray(x.astype(BF))
    return {
        "w1f": bf(w1f.reshape(128, 8 * 2 * 128)),
        "w2f": bf(w2f.reshape(128, 8 * 3 * 128)),
        "w3f": bf(w3f.reshape(128, 3 * 128)),
        "w3i": bf(w3i.reshape(128, 3 * 128)),
        "wfim": bf(wfim.reshape(128, 8 * 3 * 128)),
        "w1i": bf(w1i.reshape(128, 8 * 2 * 128)),
        "wien": bf(wien.reshape(128, 2 * 8192)),
        "ident": bf(np.eye(128, dtype=np.float32)),
    }


# ---------------------------------------------------------------- balancer
class EW:
    """Static greedy balancer for elementwise work across DVE/ACT/Pool."""

    def __init__(self, nc, pool_psum=False):
        self.nc = nc
        self.pool_psum = pool_psum
        self.load = {"v": 0.0, "a": 0.0, "p": 0.0}

    def _cost(self, e, free, two_byte, psum, kind="tt"):
        if e == "v":
            return free * 1.04 * (0.5 if two_byte else 1.0) + (130 if psum else 62) + 70
        if e == "a":
            return free * 0.833 + 185 + 57
        # Pool (Q7 software): copy at 0.6 eff, mult/add at 0.42; tensor_scalar
        # is catastrophically slow (~15us) - never placed here.
        eff = 0.6 if kind == "copy" else 0.42
        return free * 0.833 / eff + 131

    def _pick(self, allowed, free, two_byte, psum, kind="tt"):
        e = min(
            allowed,
            key=lambda x: self.load[x] + self._cost(x, free, two_byte, psum, kind),
        )
        self.load[e] += self._cost(e, free, two_byte, psum, kind)
        return e

    def evac(self, dst, src):
        free = int(np.prod(src.shape[1:]))
        two = mybir.dt.size(src.dtype) == 2 and mybir.dt.size(dst.dtype) == 2
        allowed = ["v", "a"] + (["p"] if self.pool_psum else [])
        e = self._pick(allowed, free, two, True, "copy")
        if e == "v":
            self.nc.vector.tensor_copy(dst, src)
        elif e == "a":
            self.nc.scalar.copy(dst, src)
        else:
            self.nc.gpsimd.tensor_copy(dst, src)

    def tt(self, op, dst, a, b):
        free = int(np.prod(dst.shape[1:]))
        e = self._pick(["v", "p"], free, True, False)
        eng = self.nc.vector if e == "v" else self.nc.gpsimd
        eng.tensor_tensor(dst, a, b, op)

    def sq(self, dst, src):
        # square: ACT Square or mul on DVE/Pool
        free = int(np.prod(dst.shape[1:]))
        e = self._pick(["v", "a", "p"], free, True, False)
        if e == "a":
            self.nc.scalar.activation(dst, src, AF.Square)
        else:
            eng = self.nc.vector if e == "v" else self.nc.gpsimd
            eng.tensor_tensor(dst, src, src, ALU.mult)

    def ts(self, dst, src, s0, s1, op0, op1):
        free = int(np.prod(dst.shape[1:]))
        self.load["v"] += self._cost("v", free, True, False)
        self.nc.vector.tensor_scalar(dst, src, s0, s1, op0=op0, op1=op1)

    def maxs(self, dst, src, s0):
        free = int(np.prod(dst.shape[1:]))
        self.load["v"] += self._cost("v", free, True, False)
        self.nc.vector.tensor_scalar_max(dst, src, s0)

    def act(self, dst, src, func, scale=1.0):
        free = int(np.prod(dst.shape[1:]))
        self.load["a"] += self._cost("a", free, True, False)
        self.nc.scalar.activation(dst, src, func, scale=scale)


# ---------------------------------------------------------------- device IR
def build_program(n_imgs: int = IMGS, dbg_stage: str | None = None):
    nc = bacc.Bacc("TRN2", target_bir_lowering=False, debug=False)
    y3 = nc.dram_tensor("y3", [n_imgs, N, N], F32, kind="ExternalInput")
    o3 = nc.dram_tensor("o3", [n_imgs, N, N], F32, kind="ExternalOutput")
    if dbg_stage:
        dbr = nc.dram_tensor("dbgr", [128, 8192], F32, kind="ExternalOutput")
        dbi = nc.dram_tensor("dbgi", [128, 8192], F32, kind="ExternalOutput")
    dw = {
        k: nc.dram_tensor(k, list(s), BDT, kind="ExternalInput")
        for k, s in {
            "w1f": (128, 2048),
            "w2f": (128, 3072),
            "w3f": (128, 384),
            "w3i": (128, 384),
            "wfim": (128, 3072),
            "w1i": (128, 2048),
            "wien": (128, 16384),
            "ident": (128, 128),
        }.items()
    }

    with tile.TileContext(nc) as tc:
        import contextlib

        with contextlib.ExitStack() as ctx:
            const = ctx.enter_context(tc.tile_pool(name="const", bufs=1))
            plan = ctx.enter_context(tc.tile_pool(name="plan", bufs=1))
            ypool = ctx.enter_context(tc.tile_pool(name="ypool", bufs=2))
            ps = ctx.enter_context(tc.tile_pool(name="ps", bufs=4, space="PSUM"))
            ps2 = ctx.enter_context(tc.tile_pool(name="ps2", bufs=2, space="PSUM"))
            tmp = ctx.enter_context(tc.tile_pool(name="tmp", bufs=10))
            zw = ctx.enter_context(tc.tile_pool(name="zw", bufs=3))

            ew = EW(nc, pool_psum=False)

            # constants
            sw = {}
            for k in dw:
                shp = [128, dw[k].shape[1]]
                t_ = const.tile(shp, BDT, name=k)
                nc.sync.dma_start(out=t_, in_=dw[k].ap())
                sw[k] = t_
            w1fv = sw["w1f"].rearrange("p (j ck) -> p j ck", j=8)  # ck = (c2,k1)
            w2f = sw["w2f"].rearrange("p (m c k) -> p m c k", m=8, c=3)
            w3f = sw["w3f"].rearrange("p (c k) -> p c k", c=3)
            w3iv = sw["w3i"].rearrange("p (c k) -> p c k", c=3)
            wfim = sw["wfim"].rearrange("p (m c k) -> p m c k", m=8, c=3)
            w1i = sw["w1i"].rearrange("p (j c k) -> p j c k", j=8, c=2)
            wien = sw["wien"].rearrange("p (c f) -> p c f", c=2)
            ident = sw["ident"]

            def _snap(stage, br, bi):
                if dbg_stage == stage:
                    nc.gpsimd.dma_start(out=dbr.ap(), in_=br)
                    nc.gpsimd.dma_start(out=dbi.ap(), in_=bi)

            # persistent plan buffers (bf16 [128, 8192] each; B planes merged
            # so stage-A can evacuate re+im in one instruction)
            Ar = plan.tile([128, 8192], BDT, name="Ar")
            Ai = plan.tile([128, 8192], BDT, name="Ai")
            Bri = plan.tile([128, 16384], BDT, name="Bri")
            Br = Bri[:, :8192]
            Bi = Bri[:, 8192:]

            for img in range(n_imgs):
                # ---- load (fp32 HBM -> bf16 SBUF, SWDGE cast)
                y_t = ypool.tile([128, 8192], BDT)
                nc.gpsimd.dma_start(
                    out=y_t, in_=y3.ap()[img].rearrange("(p j) w -> p (j w)", j=8)
                )

                # ---- stage A (fused S1+T1): y blocks stationary, W1_j moving
                # out B[p=b, f = 1024m + 128j + k1] (complex)
                yv = y_t.rearrange("p (j b m) -> p j m b", j=8, b=128)
                for m in range(8):
                    for g in range(2):  # j groups of 4 -> one [128,1024] psum tile
                        pt = ps2.tile([128, 1024], F32, tag="p2")
                        for jj in range(4):
                            j = 4 * g + jj
                            nc.tensor.matmul(
                                pt[:, 256 * jj : 256 * (jj + 1)],
                                yv[:, j, m],
                                w1fv[:, j],
                                start=True,
                                stop=True,
                            )
                        ptv = pt.rearrange("p (jj c k) -> p c jj k", jj=4, c=2)
                        dst = Bri.rearrange(
                            "p (pl mm gg jj k) -> p mm gg pl jj k",
                            pl=2, mm=8, gg=2, jj=4,
                        )[:, m, g]
                        ew.evac(dst, ptv)

                _snap("A", Br, Bi)
                # ---- S2: per m contract b -> A[p=kb, f=interleaved]
                for m in range(8):
                    for c in range(2):
                        jr = Br[:, 1024 * m + 512 * c : 1024 * m + 512 * (c + 1)]
                        ji = Bi[:, 1024 * m + 512 * c : 1024 * m + 512 * (c + 1)]
                        pr = ps.tile([128, 512], F32, tag="pp")
                        pi = ps.tile([128, 512], F32, tag="pp")
                        nc.tensor.matmul(pr, w2f[:, m, 0], jr, start=True, stop=False)
                        nc.tensor.matmul(pi, w2f[:, m, 1], jr, start=True, stop=False)
                        nc.tensor.matmul(pr, w2f[:, m, 2], ji, start=False, stop=True)
                        nc.tensor.matmul(pi, w2f[:, m, 0], ji, start=False, stop=True)
                        # interleaved evac: psum (jj,q,t) -> f = 128q + 16m + 8c + 2jj + t
                        for dstp, srcp in ((Ar, pr), (Ai, pi)):
                            sview = srcp.rearrange("p (jj q t) -> p jj q t", jj=4, q=64)
                            dview = dstp.rearrange(
                                "p (q mm cc jj t) -> p q mm cc jj t",
                                q=64, mm=8, cc=2, jj=4,
                            )[:, :, m, c, :, :].transpose([0, 2, 1, 3])
                            ew.evac(dview, sview)

                _snap("S2", Ar, Ai)
                # ---- T2: A -> B[p=g_in=(m,j,t), f=(q,kb)]
                for g in range(8):  # groups of 8 q
                    for src, dst in ((Ar, Br), (Ai, Bi)):
                        pt = ps.tile([128, 1024], BDT, tag="pp")
                        for qq in range(8):
                            q = 8 * g + qq
                            nc.tensor.transpose(
                                pt[:, 128 * qq : 128 * (qq + 1)],
                                src[:, 128 * q : 128 * (q + 1)],
                                ident,
                            )
                        ew.evac(dst[:, 1024 * g : 1024 * (g + 1)], pt)

                _snap("T2", Br, Bi)
                # ---- S3 + Wiener + fused S3'+T2' : B -> A[p=kb, f=(q,g2)]
                nch = 8192 // CH
                for c in range(nch):
                    sl = slice(CH * c, CH * (c + 1))
                    zri = zw.tile([128, 2 * CH], BDT, tag="zri")
                    zr = zri[:, :CH]
                    zi = zri[:, CH:]
                    for hh in range(CH // 512):
                        bsl = slice(CH * c + 512 * hh, CH * c + 512 * (hh + 1))
                        pt = ps2.tile([128, 1024], F32, tag="p2")
                        pr = pt[:, :512]
                        pi = pt[:, 512:]
                        nc.tensor.matmul(pr, w3f[:, 0], Br[:, bsl], start=True, stop=False)
                        nc.tensor.matmul(pi, w3f[:, 1], Br[:, bsl], start=True, stop=False)
                        nc.tensor.matmul(pr, w3f[:, 2], Bi[:, bsl], start=False, stop=True)
                        nc.tensor.matmul(pi, w3f[:, 0], Bi[:, bsl], start=False, stop=True)
                        dz = zri.rearrange("p (pl h k) -> p h pl k", pl=2, h=CH // 512)[:, hh]
                        ew.evac(dz, pt.rearrange("p (pl k) -> p pl k", pl=2))
                    rrc = wien[:, 0, sl]
                    ric = wien[:, 1, sl]
                    t1 = tmp.tile([128, CH], BDT, tag="wt")
                    ew.sq(t1, zr)
                    t2 = tmp.tile([128, CH], BDT, tag="wt")
                    ew.sq(t2, zi)
                    mm_ = tmp.tile([128, CH], BDT, tag="wt")
                    ew.tt(ALU.add, mm_, t1, t2)
                    rc = tmp.tile([128, CH], BDT, tag="wt")
                    ew.ts(rc, mm_, CSUB, 0.0, ALU.subtract, ALU.max)
                    u2 = tmp.tile([128, CH], F32, tag="wtf", bufs=2)
                    ew.maxs(u2, mm_, CSUB)
                    ln_ = tmp.tile([128, CH], F32, tag="wtf", bufs=2)
                    ew.act(ln_, u2, AF.Ln)
                    r_ = tmp.tile([128, CH], BDT, tag="wt")
                    ew.act(r_, ln_, AF.Exp, scale=-1.0)
                    w_ = tmp.tile([128, CH], BDT, tag="wt")
                    ew.tt(ALU.mult, w_, rc, r_)
                    fr = tmp.tile([128, CH], BDT, tag="wt")
                    ew.tt(ALU.mult, fr, w_, rrc)
                    fi = tmp.tile([128, CH], BDT, tag="wt")
                    ew.tt(ALU.mult, fi, w_, ric)
                    p1 = tmp.tile([128, CH], BDT, tag="wt")
                    ew.tt(ALU.mult, p1, zr, fr)
                    p2 = tmp.tile([128, CH], BDT, tag="wt")
                    ew.tt(ALU.mult, p2, zi, fi)
                    zwr = zw.tile([128, CH], BDT, tag="zwr")
                    ew.tt(ALU.subtract, zwr, p1, p2)
                    p3 = tmp.tile([128, CH], BDT, tag="wt")
                    ew.tt(ALU.mult, p3, zr, fi)
                    p4 = tmp.tile([128, CH], BDT, tag="wt")
                    ew.tt(ALU.mult, p4, zi, fr)
                    zwi = zw.tile([128, CH], BDT, tag="zwi")
                    ew.tt(ALU.add, zwi, p3, p4)
                    # fused S3'+T2': zw blocks stationary, conj(W3) moving
                    ptr = ps2.tile([128, CH], F32, tag="p2")
                    pti = ps2.tile([128, CH], F32, tag="p2")
                    for qq in range(CH // 128):
                        qsl = slice(128 * qq, 128 * (qq + 1))
                        nc.tensor.matmul(ptr[:, qsl], zwr[:, qsl], w3iv[:, 0], start=True, stop=False)
                        nc.tensor.matmul(pti[:, qsl], zwr[:, qsl], w3iv[:, 1], start=True, stop=False)
                        nc.tensor.matmul(ptr[:, qsl], zwi[:, qsl], w3iv[:, 2], start=False, stop=True)
                        nc.tensor.matmul(pti[:, qsl], zwi[:, qsl], w3iv[:, 0], start=False, stop=True)
                    ew.evac(Ar[:, sl], ptr)
                    ew.evac(Ai[:, sl], pti)

                _snap("S3p", Ar, Ai)
                # ---- S2': per m contract kb (twiddle-folded conj(F128)) -> B[p=b, f=(q,t,j,m)]
                Avr = Ar.rearrange("p (q t j m) -> p q t j m", q=64, t=2, j=8)
                Avi = Ai.rearrange("p (q t j m) -> p q t j m", q=64, t=2, j=8)
                Bvr = Br.rearrange("p (q t j m) -> p q t j m", q=64, t=2, j=8)
                Bvi = Bi.rearrange("p (q t j m) -> p q t j m", q=64, t=2, j=8)
                for m in range(8):
                    for c in range(2):
                        qsl = slice(32 * c, 32 * (c + 1))
                        jr = Avr[:, qsl, :, :, m]
                        ji = Avi[:, qsl, :, :, m]
                        pr = ps.tile([128, 512], F32, tag="pp")
                        pi = ps.tile([128, 512], F32, tag="pp")
                        nc.tensor.matmul(pr, wfim[:, m, 0], jr, start=True, stop=False)
                        nc.tensor.matmul(pi, wfim[:, m, 1], jr, start=True, stop=False)
                        nc.tensor.matmul(pr, wfim[:, m, 2], ji, start=False, stop=True)
                        nc.tensor.matmul(pi, wfim[:, m, 0], ji, start=False, stop=True)
                        prv = pr.rearrange("p (q t j) -> p q t j", q=32, t=2)
                        piv = pi.rearrange("p (q t j) -> p q t j", q=32, t=2)
                        ew.evac(Bvr[:, qsl, :, :, m], prv)
                        ew.evac(Bvi[:, qsl, :, :, m], piv)

                _snap("S2p", Br, Bi)
                # ---- T1': B[p=b, f=(q,t,j,m)] -> A[p=k1, f=(j,m,b)]
                vB4r = Br.rearrange("p (q t j m) -> p j m (q t)", q=64, t=2, j=8)
                vB4i = Bi.rearrange("p (q t j m) -> p j m (q t)", q=64, t=2, j=8)
                for j in range(8):
                    for src, dst in ((vB4r, Ar), (vB4i, Ai)):
                        pt = ps.tile([128, 1024], BDT, tag="pp")
                        for mj in range(8):
                            nc.tensor.transpose(
                                pt[:, 128 * mj : 128 * (mj + 1)], src[:, j, mj], ident
                            )
                        ew.evac(dst[:, 1024 * j : 1024 * (j + 1)], pt)

                _snap("T1p", Ar, Ai)
                # ---- S1': per j contract k1 (real out) -> out[p=a, f=(j, 8b+m)]
                out_t = ypool.tile([128, 8192], BDT, tag="y_t")
                for j in range(8):
                    for c in range(2):
                        off = 1024 * j + 512 * c
                        jr = Ar[:, off : off + 512]
                        ji = Ai[:, off : off + 512]
                        pr = ps.tile([128, 512], F32, tag="pp")
                        nc.tensor.matmul(pr, w1i[:, j, 0], jr, start=True, stop=False)
                        nc.tensor.matmul(pr, w1i[:, j, 1], ji, start=False, stop=True)
                        # evac with digit swap (m,b) -> 8b+m
                        src = pr.rearrange("p (m b) -> p m b", m=4)
                        dst = out_t.rearrange("p (j b m) -> p j b m", j=8, b=128)[
                            :, j, :, 4 * c : 4 * (c + 1)
                        ].transpose([0, 2, 1])
                        ew.evac(dst, src)
                _snap("OUT", out_t, out_t)
                nc.gpsimd.dma_start(
                    out=o3.ap()[img].rearrange("(p j) w -> p (j w)", j=8), in_=out_t
                )

    nc.compile()
    return nc


_PROG = None


def _get_prog():
    global _PROG
    if _PROG is None:
        _PROG = build_program(IMGS)
    return _PROG


def kernel(y: np.ndarray, psf: np.ndarray) -> np.ndarray:
    consts = _host_consts(np.asarray(psf, np.float64)[0, 0])
    nc = _get_prog()
    y24 = np.ascontiguousarray(np.asarray(y, np.float32).reshape(N_CORES * IMGS, N, N))
    in_maps = []
    for c in range(N_CORES):
        m = dict(consts)
        m["y3"] = y24[IMGS * c : IMGS * (c + 1)]
        in_maps.append(m)
    res = run_bass_kernel_spmd(nc, in_maps, core_ids=list(range(N_CORES)))
    out = np.stack([res.results[c]["o3"] for c in range(N_CORES)])
    return out.reshape(8, 3, N, N).astype(np.float32)


# revision 16
# speedup vs baseline: 1.1948x; 1.0414x over previous
"""BM3D-deblur (regularized-inverse + global empirical Wiener) on 8 Trainium2 cores.

Math (per 1024x1024 image-channel, 24 total, 3 per core):
  G = fft2(y); Z = G*ri; S = max(|Z|^2/n - psd, 0); Wf = S/(S+psd+eps)
  out = real(ifft2(Z*Wf))
with ri, psd derived from the 25x25 PSF on the host (tiny).

2D FFT via digit decomposition h = 8a+j, w = 8b+m, k_h = (2q+t)+128*kj,
k_w = kb+128*km. Pipeline (planes A/B alternate, all [128,8192] bf16):
  stageA: fused S1+T1 - y-blocks as stationary (lhsT), W1_j moving
          -> B[p=b, f=(m,j,k1)]  (contracts a, transposes b to partitions)
  S2:     per m contract b with F128*tw_m        -> A[p=kb, f=interleave]
  T2:     PE transpose                           -> B[p=(m,j,t), f=(q,kb)]
  S3:     contract (j,m) with D8xD8              -> zr/zi chunks
  Wiener: elementwise (chunked, 3-engine balanced)
  S3'+T2' fused: zw-blocks as stationary, conj(W3) moving
          -> A[p=kb, f=(q,g2)]   (contracts spec, transposes kb up)
  S2':    per m contract kb with conj(F128)*exp(2pi i m kb/N) (twiddle
          folded into 8 per-m matrices)          -> B[p=b, f=(q,t,j,m)]
  T1':    PE transpose                           -> A[p=k1, f=(j,m,b)]
  S1':    per j contract k1 (real out)           -> out[p=a, f=(j,8b+m)]
All spectral coefficient planes permuted on host into device layout.
Elementwise work (PSUM evacs + Wiener) is spread across DVE/ACT/Pool by a
static greedy balancer; ACT uses the reciprocal_and_small table (Square,
Reciprocal, Copy - no table reloads).
"""
import sys

sys.path.insert(0, "/opt/trn_rl_repo")

import numpy as np
import ml_dtypes

import concourse.bass as bass
import concourse.bacc as bacc
import concourse.tile as tile
from concourse import mybir
from concourse.bass_utils import run_bass_kernel_spmd
import concourse.hw_specs as _hw_specs

_orig_get_tables = _hw_specs.get_activation_tables


def _patched_tables(arch):
    t = dict(_orig_get_tables(arch))
    pin = {
        mybir.ActivationFunctionType.Ln,
        mybir.ActivationFunctionType.Exp,
        mybir.ActivationFunctionType.Square,
    }
    for name in list(t):
        if name != "natural_log_exp_and_others" and (t[name] & pin):
            t[name] = t[name] - pin
    return t


bacc.get_activation_tables = _patched_tables

N = 1024
SIGMA = 0.05
CSUB = float(SIGMA**2 * N * N * N * N)  # psd = sigma^2 * n^2 * a
N_CORES = 8
IMGS = 3  # images per core
CH = 1024  # wiener chunk cols

BDT = mybir.dt.bfloat16
F32 = mybir.dt.float32
BF = ml_dtypes.bfloat16
AF = mybir.ActivationFunctionType
ALU = mybir.AluOpType


# ---------------------------------------------------------------- host math
def _host_consts(psf25: np.ndarray) -> dict[str, np.ndarray]:
    ar = np.arange(128)
    F128 = np.exp(-2j * np.pi * np.outer(ar, ar) / 128)
    D8 = np.exp(-2j * np.pi * np.outer(np.arange(8), np.arange(8)) / 8)
    tw = np.exp(-2j * np.pi * np.outer(np.arange(8), ar) / N)  # [j, k1]

    # forward W1 per j (moving operand of fused stage-A): [a, (j, c2, k1)]
    w1f = np.empty((128, 8, 2, 128), np.float32)
    for j in range(8):
        Wj = F128 * tw[j][None, :]
        w1f[:, j, 0] = Wj.real
        w1f[:, j, 1] = Wj.imag
    # w2f: same matrices, planes (re, im, -im)
    w2f = np.empty((128, 8, 3, 128), np.float32)
    for m in range(8):
        Wm = F128 * tw[m][None, :]
        w2f[:, m, 0] = Wm.real
        w2f[:, m, 1] = Wm.imag
        w2f[:, m, 2] = -Wm.imag
    # W3 fwd: rows g_in=16m+2j+t (T2 col enum), cols spec p=16kj+2km+t
    # W3i inv: rows spec p, cols g2=64t+8j+m
    W3 = np.zeros((128, 128), complex)
    W3i = np.zeros((128, 128), complex)
    for j in range(8):
        for m in range(8):
            for kj in range(8):
                for km in range(8):
                    v = D8[j, kj] * D8[m, km]
                    for t in range(2):
                        W3[16 * m + 2 * j + t, 16 * kj + 2 * km + t] = v
                        W3i[16 * kj + 2 * km + t, 64 * t + 8 * j + m] = np.conj(v)
    w3f = np.stack([W3.real, W3.imag, -W3.imag], 1).astype(np.float32)  # [128,3,128]
    w3i = np.stack([W3i.real, W3i.imag, -W3i.imag], 1).astype(np.float32)
    # inverse S2' lhsT per m: [kb, b] = conj(F128) * exp(+2pi i m kb / N)
    # (inverse W-axis twiddle folded in); planes (re, im, -im)
    wfim = np.empty((128, 8, 3, 128), np.float32)
    kb = np.arange(128)
    for m in range(8):
        Vm = np.conj(F128) * np.exp(2j * np.pi * m * kb / N)[:, None]
        wfim[:, m, 0] = Vm.real
        wfim[:, m, 1] = Vm.imag
        wfim[:, m, 2] = -Vm.imag
    # inverse S1' lhsT per j: [k1, a] = conj(W1_j).T ; planes (re, -im) (real out)
    w1i = np.empty((128, 8, 2, 128), np.float32)
    for j in range(8):
        V = np.conj(F128 * tw[j][None, :]).T
        w1i[:, j, 0] = V.real
        w1i[:, j, 1] = -V.imag
    # wiener planes in spectral device layout
    P = np.zeros((N, N))
    P[:25, :25] = psf25
    P = np.roll(P, (-12, -12), axis=(0, 1))
    Hf = np.fft.fft2(P)
    ri = np.conj(Hf) / (np.abs(Hf) ** 2 + SIGMA**2)
    p = np.arange(128)
    kj, km, t = p // 16, (p % 16) // 2, p % 2
    f = np.arange(8192)
    q, kbf = f // 128, f % 128
    kh = (2 * q[None, :] + t[:, None]) + 128 * kj[:, None]
    kw = kbf[None, :] + 128 * km[:, None]
    rr_dev = (ri.real / (N * N))[kh, kw]
    rii_dev = (ri.imag / (N * N))[kh, kw]
    wien = np.stack([rr_dev, rii_dev], 1).astype(np.float32)  # [128,2,8192]

    bf = lambda x: np.ascontiguousarray(x.astype(BF))
    return {
        "w1f": bf(w1f.reshape(128, 8 * 2 * 128)),
        "w2f": bf(w2f.reshape(128, 8 * 3 * 128)),
        "w3f": bf(w3f.reshape(128, 3 * 128)),
        "w3i": bf(w3i.reshape(128, 3 * 128)),
        "wfim": bf(wfim.reshape(128, 8 * 3 * 128)),
        "w1i": bf(w1i.reshape(128, 8 * 2 * 128)),
        "wien": bf(wien.reshape(128, 2 * 8192)),
        "ident": bf(np.eye(128, dtype=np.float32)),
    }


# ---------------------------------------------------------------- balancer
class EW:
    """Static greedy balancer for elementwise work across DVE/ACT/Pool."""

    def __init__(self, nc, pool_psum=False):
        self.nc = nc
        self.pool_psum = pool_psum
        self.load = {"v": 0.0, "a": 0.0, "p": 0.0}

    def _cost(self, e, free, two_byte, psum, kind="tt"):
        if e == "v":
            return free * 1.04 * (0.5 if two_byte else 1.0) + (130 if psum else 62) + 70
        if e == "a":
            return free * 0.833 + 185 + 57
        # Pool (Q7 software): copy at 0.6 eff, mult/add at 0.42; tensor_scalar
        # is catastrophically slow (~15us) - never placed here.
        eff = 0.6 if kind == "copy" else 0.42
        return free * 0.833 / eff + 131

    def _pick(self, allowed, free, two_byte, psum, kind="tt"):
        e = min(
            allowed,
            key=lambda x: self.load[x] + self._cost(x, free, two_byte, psum, kind),
        )
        self.load[e] += self._cost(e, free, two_byte, psum, kind)
        return e

    def evac(self, dst, src):
        free = int(np.prod(src.shape[1:]))
        two = mybir.dt.size(src.dtype) == 2 and mybir.dt.size(dst.dtype) == 2
        allowed = ["v", "a"] + (["p"] if self.pool_psum else [])
        e = self._pick(allowed, free, two, True, "copy")
        if e == "v":
            self.nc.vector.tensor_copy(dst, src)
        elif e == "a":
            self.nc.scalar.copy(dst, src)
        else:
            self.nc.gpsimd.tensor_copy(dst, src)

    def tt(self, op, dst, a, b):
        free = int(np.prod(dst.shape[1:]))
        e = self._pick(["v", "p"], free, True, False)
        eng = self.nc.vector if e == "v" else self.nc.gpsimd
        eng.tensor_tensor(dst, a, b, op)

    def sq(self, dst, src):
        # square: ACT Square or mul on DVE/Pool
        free = int(np.prod(dst.shape[1:]))
        e = self._pick(["v", "a", "p"], free, True, False)
        if e == "a":
            self.nc.scalar.activation(dst, src, AF.Square)
        else:
            eng = self.nc.vector if e == "v" else self.nc.gpsimd
            eng.tensor_tensor(dst, src, src, ALU.mult)

    def ts(self, dst, src, s0, s1, op0, op1):
        free = int(np.prod(dst.shape[1:]))
        self.load["v"] += self._cost("v", free, True, False)
        self.nc.vector.tensor_scalar(dst, src, s0, s1, op0=op0, op1=op1)

    def maxs(self, dst, src, s0):
        free = int(np.prod(dst.shape[1:]))
        self.load["v"] += self._cost("v", free, True, False)
        self.nc.vector.tensor_scalar_max(dst, src, s0)

    def act(self, dst, src, func, scale=1.0):
        free = int(np.prod(dst.shape[1:]))
        self.load["a"] += self._cost("a", free, True, False)
        self.nc.scalar.activation(dst, src, func, scale=scale)


# ---------------------------------------------------------------- device IR
def build_program(n_imgs: int = IMGS, dbg_stage: str | None = None):
    nc = bacc.Bacc("TRN2", target_bir_lowering=False, debug=False)
    y3 = nc.dram_tensor("y3", [n_imgs, N, N], F32, kind="ExternalInput")
    o3 = nc.dram_tensor("o3", [n_imgs, N, N], F32, kind="ExternalOutput")
    if dbg_stage:
        dbr = nc.dram_tensor("dbgr", [128, 8192], F32, kind="ExternalOutput")
        dbi = nc.dram_tensor("dbgi", [128, 8192], F32, kind="ExternalOutput")
    dw = {
        k: nc.dram_tensor(k, list(s), BDT, kind="ExternalInput")
        for k, s in {
            "w1f": (128, 2048),
            "w2f": (128, 3072),
            "w3f": (128, 384),
            "w3i": (128, 384),
            "wfim": (128, 3072),
            "w1i": (128, 2048),
            "wien": (128, 16384),
            "ident": (128, 128),
        }.items()
    }

    with tile.TileContext(nc) as tc:
        import contextlib

        with contextlib.ExitStack() as ctx:
            const = ctx.enter_context(tc.tile_pool(name="const", bufs=1))
            plan = ctx.enter_context(tc.tile_pool(name="plan", bufs=1))
            ypool = ctx.enter_context(tc.tile_pool(name="ypool", bufs=2))
            ps = ctx.enter_context(tc.tile_pool(name="ps", bufs=4, space="PSUM"))
            tmp = ctx.enter_context(tc.tile_pool(name="tmp", bufs=10))
            zw = ctx.enter_context(tc.tile_pool(name="zw", bufs=3))

            ew = EW(nc, pool_psum=False)

            # constants
            sw = {}
            for k in dw:
                shp = [128, dw[k].shape[1]]
                t_ = const.tile(shp, BDT, name=k)
                nc.sync.dma_start(out=t_, in_=dw[k].ap())
                sw[k] = t_
            w1fv = sw["w1f"].rearrange("p (j ck) -> p j ck", j=8)  # ck = (c2,k1)
            w2f = sw["w2f"].rearrange("p (m c k) -> p m c k", m=8, c=3)
            w3f = sw["w3f"].rearrange("p (c k) -> p c k", c=3)
            w3iv = sw["w3i"].rearrange("p (c k) -> p c k", c=3)
            wfim = sw["wfim"].rearrange("p (m c k) -> p m c k", m=8, c=3)
            w1i = sw["w1i"].rearrange("p (j c k) -> p j c k", j=8, c=2)
            wien = sw["wien"].rearrange("p (c f) -> p c f", c=2)
            ident = sw["ident"]

            def _snap(stage, br, bi):
                if dbg_stage == stage:
                    nc.gpsimd.dma_start(out=dbr.ap(), in_=br)
                    nc.gpsimd.dma_start(out=dbi.ap(), in_=bi)

            # persistent plan buffers (bf16 [128, 8192] each; B planes merged
            # so stage-A can evacuate re+im in one instruction)
            Ari = plan.tile([128, 16384], BDT, name="Ari")
            Ar = Ari[:, :8192]
            Ai = Ari[:, 8192:]
            Bri = plan.tile([128, 16384], BDT, name="Bri")
            Br = Bri[:, :8192]
            Bi = Bri[:, 8192:]

            for img in range(n_imgs):
                # ---- load (fp32 HBM -> bf16 SBUF, SWDGE cast)
                y_t = ypool.tile([128, 8192], BDT)
                nc.gpsimd.dma_start(
                    out=y_t, in_=y3.ap()[img].rearrange("(p j) w -> p (j w)", j=8)
                )

                # ---- stage A (fused S1+T1): y blocks stationary, W1_j moving
                # out B[p=b, f = 1024m + 128j + k1] (complex)
                yv = y_t.rearrange("p (j b m) -> p j m b", j=8, b=128)
                for m in range(8):
                    for g in range(2):  # j groups of 4 -> one [128,1024] psum tile
                        pt = ps.tile([128, 1024], F32, tag="pp")
                        for jj in range(4):
                            j = 4 * g + jj
                            nc.tensor.matmul(
                                pt[:, 256 * jj : 256 * (jj + 1)],
                                yv[:, j, m],
                                w1fv[:, j],
                                start=True,
                                stop=True,
                            )
                        ptv = pt.rearrange("p (jj c k) -> p c jj k", jj=4, c=2)
                        dst = Bri.rearrange(
                            "p (pl mm gg jj k) -> p mm gg pl jj k",
                            pl=2, mm=8, gg=2, jj=4,
                        )[:, m, g]
                        ew.evac(dst, ptv)

                _snap("A", Br, Bi)
                # ---- S2: per m contract b -> A[p=kb, f=interleaved]
                for m in range(8):
                    for c in range(2):
                        jr = Br[:, 1024 * m + 512 * c : 1024 * m + 512 * (c + 1)]
                        ji = Bi[:, 1024 * m + 512 * c : 1024 * m + 512 * (c + 1)]
                        prc = ps.tile([128, 1024], F32, tag="pp")
                        pr = prc[:, :512]
                        pi = prc[:, 512:]
                        nc.tensor.matmul(pr, w2f[:, m, 0], jr, start=True, stop=False)
                        nc.tensor.matmul(pi, w2f[:, m, 1], jr, start=True, stop=False)
                        nc.tensor.matmul(pr, w2f[:, m, 2], ji, start=False, stop=True)
                        nc.tensor.matmul(pi, w2f[:, m, 0], ji, start=False, stop=True)
                        # interleaved evac: psum (jj,q,t) -> f = 128q + 16m + 8c + 2jj + t
                        for dstp, srcp in ((Ar, pr), (Ai, pi)):
                            sview = srcp.rearrange("p (jj q t) -> p jj q t", jj=4, q=64)
                            dview = dstp.rearrange(
                                "p (q mm cc jj t) -> p q mm cc jj t",
                                q=64, mm=8, cc=2, jj=4,
                            )[:, :, m, c, :, :].transpose([0, 2, 1, 3])
                            ew.evac(dview, sview)

                _snap("S2", Ar, Ai)
                # ---- T2: A -> B[p=g_in=(m,j,t), f=(q,kb)]
                for g in range(8):  # groups of 8 q
                    for srcv, dst in ((Ar, Br), (Ai, Bi)):
                        pt = ps.tile([128, 1024], BDT, tag="pp")
                        for qq in range(8):
                            q = 8 * g + qq
                            nc.tensor.transpose(
                                pt[:, 128 * qq : 128 * (qq + 1)],
                                srcv[:, 128 * q : 128 * (q + 1)],
                                ident,
                            )
                        ew.evac(dst[:, 1024 * g : 1024 * (g + 1)], pt)

                _snap("T2", Br, Bi)
                # ---- S3 + Wiener + fused S3'+T2' : B -> A[p=kb, f=(q,g2)]
                nch = 8192 // CH
                for c in range(nch):
                    sl = slice(CH * c, CH * (c + 1))
                    zri = zw.tile([128, 2 * CH], BDT, tag="zri")
                    zr = zri[:, :CH]
                    zi = zri[:, CH:]
                    for hh in range(CH // 512):
                        bsl = slice(CH * c + 512 * hh, CH * c + 512 * (hh + 1))
                        pt = ps.tile([128, 1024], F32, tag="pp")
                        pr = pt[:, :512]
                        pi = pt[:, 512:]
                        nc.tensor.matmul(pr, w3f[:, 0], Br[:, bsl], start=True, stop=False)
                        nc.tensor.matmul(pi, w3f[:, 1], Br[:, bsl], start=True, stop=False)
                        nc.tensor.matmul(pr, w3f[:, 2], Bi[:, bsl], start=False, stop=True)
                        nc.tensor.matmul(pi, w3f[:, 0], Bi[:, bsl], start=False, stop=True)
                        dz = zri.rearrange("p (pl h k) -> p h pl k", pl=2, h=CH // 512)[:, hh]
                        ew.evac(dz, pt.rearrange("p (pl k) -> p pl k", pl=2))
                    rrc = wien[:, 0, sl]
                    ric = wien[:, 1, sl]
                    t1 = tmp.tile([128, CH], BDT, tag="wt")
                    ew.sq(t1, zr)
                    t2 = tmp.tile([128, CH], BDT, tag="wt")
                    ew.sq(t2, zi)
                    mm_ = tmp.tile([128, CH], BDT, tag="wt")
                    ew.tt(ALU.add, mm_, t1, t2)
                    rc = tmp.tile([128, CH], BDT, tag="wt")
                    ew.ts(rc, mm_, CSUB, 0.0, ALU.subtract, ALU.max)
                    u2 = tmp.tile([128, CH], F32, tag="wtf", bufs=2)
                    ew.maxs(u2, mm_, CSUB)
                    ln_ = tmp.tile([128, CH], F32, tag="wtf", bufs=2)
                    ew.act(ln_, u2, AF.Ln)
                    r_ = tmp.tile([128, CH], BDT, tag="wt")
                    ew.act(r_, ln_, AF.Exp, scale=-1.0)
                    w_ = tmp.tile([128, CH], BDT, tag="wt")
                    ew.tt(ALU.mult, w_, rc, r_)
                    fr = tmp.tile([128, CH], BDT, tag="wt")
                    ew.tt(ALU.mult, fr, w_, rrc)
                    fi = tmp.tile([128, CH], BDT, tag="wt")
                    ew.tt(ALU.mult, fi, w_, ric)
                    p1 = tmp.tile([128, CH], BDT, tag="wt")
                    ew.tt(ALU.mult, p1, zr, fr)
                    p2 = tmp.tile([128, CH], BDT, tag="wt")
                    ew.tt(ALU.mult, p2, zi, fi)
                    zwr = zw.tile([128, CH], BDT, tag="zwr")
                    ew.tt(ALU.subtract, zwr, p1, p2)
                    p3 = tmp.tile([128, CH], BDT, tag="wt")
                    ew.tt(ALU.mult, p3, zr, fi)
                    p4 = tmp.tile([128, CH], BDT, tag="wt")
                    ew.tt(ALU.mult, p4, zi, fr)
                    zwi = zw.tile([128, CH], BDT, tag="zwi")
                    ew.tt(ALU.add, zwi, p3, p4)
                    # fused S3'+T2': zw blocks stationary, conj(W3) moving
                    ptr = ps.tile([128, CH], F32, tag="pp")
                    pti = ps.tile([128, CH], F32, tag="pp")
                    for qq in range(CH // 128):
                        qsl = slice(128 * qq, 128 * (qq + 1))
                        nc.tensor.matmul(ptr[:, qsl], zwr[:, qsl], w3iv[:, 0], start=True, stop=False)
                        nc.tensor.matmul(pti[:, qsl], zwr[:, qsl], w3iv[:, 1], start=True, stop=False)
                        nc.tensor.matmul(ptr[:, qsl], zwi[:, qsl], w3iv[:, 2], start=False, stop=True)
                        nc.tensor.matmul(pti[:, qsl], zwi[:, qsl], w3iv[:, 0], start=False, stop=True)
                    ew.evac(Ar[:, sl], ptr)
                    ew.evac(Ai[:, sl], pti)

                _snap("S3p", Ar, Ai)
                # ---- S2': per m contract kb (twiddle-folded conj(F128)) -> B[p=b, f=(q,t,j,m)]
                Avr = Ar.rearrange("p (q t j m) -> p q t j m", q=64, t=2, j=8)
                Avi = Ai.rearrange("p (q t j m) -> p q t j m", q=64, t=2, j=8)
                for m in range(8):
                    for c in range(2):
                        qsl = slice(32 * c, 32 * (c + 1))
                        jr = Avr[:, qsl, :, :, m]
                        ji = Avi[:, qsl, :, :, m]
                        prc = ps.tile([128, 1024], F32, tag="pp")
                        pr = prc[:, :512]
                        pi = prc[:, 512:]
                        nc.tensor.matmul(pr, wfim[:, m, 0], jr, start=True, stop=False)
                        nc.tensor.matmul(pi, wfim[:, m, 1], jr, start=True, stop=False)
                        nc.tensor.matmul(pr, wfim[:, m, 2], ji, start=False, stop=True)
                        nc.tensor.matmul(pi, wfim[:, m, 0], ji, start=False, stop=True)
                        dst = Bri.rearrange(
                            "p (pl mm cc k) -> p mm cc pl k", pl=2, mm=8, cc=2
                        )[:, m, c]
                        ew.evac(dst, prc.rearrange("p (pl k) -> p pl k", pl=2))

                _snap("S2p", Br, Bi)
                # ---- T1': B[p=b, f=(m,c,q32,t,j)] -> A[p=k1, f=(j,m,b)]
                vB4r = Br.rearrange(
                    "p (mm cc q32 t j) -> p j mm cc q32 t", mm=8, cc=2, q32=32, t=2
                )
                vB4i = Bi.rearrange(
                    "p (mm cc q32 t j) -> p j mm cc q32 t", mm=8, cc=2, q32=32, t=2
                )
                for j in range(8):
                    for srcv, dst in ((vB4r, Ar), (vB4i, Ai)):
                        pt = ps.tile([128, 1024], BDT, tag="pp")
                        for mj in range(8):
                            nc.tensor.transpose(
                                pt[:, 128 * mj : 128 * (mj + 1)], srcv[:, j, mj], ident
                            )
                        ew.evac(dst[:, 1024 * j : 1024 * (j + 1)], pt)

                _snap("T1p", Ar, Ai)
                # ---- S1': per j contract k1 (real out) -> out[p=a, f=(j, 8b+m)]
                out_t = ypool.tile([128, 8192], BDT, tag="y_t")
                for j in range(8):
                    for c in range(2):
                        off = 1024 * j + 512 * c
                        jr = Ar[:, off : off + 512]
                        ji = Ai[:, off : off + 512]
                        pr = ps.tile([128, 512], F32, tag="pp")
                        nc.tensor.matmul(pr, w1i[:, j, 0], jr, start=True, stop=False)
                        nc.tensor.matmul(pr, w1i[:, j, 1], ji, start=False, stop=True)
                        # evac with digit swap (m,b) -> 8b+m
                        src = pr.rearrange("p (m b) -> p m b", m=4)
                        dst = out_t.rearrange("p (j b m) -> p j b m", j=8, b=128)[
                            :, j, :, 4 * c : 4 * (c + 1)
                        ].transpose([0, 2, 1])
                        ew.evac(dst, src)
                _snap("OUT", out_t, out_t)
                nc.gpsimd.dma_start(
                    out=o3.ap()[img].rearrange("(p j) w -> p (j w)", j=8), in_=out_t
                )

    nc.compile()
    return nc


_PROG = None


def _get_prog():
    global _PROG
    if _PROG is None:
        _PROG = build_program(IMGS)
    return _PROG


def kernel(y: np.ndarray, psf: np.ndarray) -> np.ndarray:
    consts = _host_consts(np.asarray(psf, np.float64)[0, 0])
    nc = _get_prog()
    y24 = np.ascontiguousarray(np.asarray(y, np.float32).reshape(N_CORES * IMGS, N, N))
    in_maps = []
    for c in range(N_CORES):
        m = dict(consts)
        m["y3"] = y24[IMGS * c : IMGS * (c + 1)]
        in_maps.append(m)
    res = run_bass_kernel_spmd(nc, in_maps, core_ids=list(range(N_CORES)))
    out = np.stack([res.results[c]["o3"] for c in range(N_CORES)])
    return out.reshape(8, 3, N, N).astype(np.float32)


# revision 18
# speedup vs baseline: 1.6691x; 1.3969x over previous
"""BM3D-deblur (regularized-inverse + global empirical Wiener) on 8 Trainium2 cores.

Math (per 1024x1024 image-channel, 24 total, 3 per core):
  G = fft2(y); Z = G*ri; S = max(|Z|^2/n - psd, 0); Wf = S/(S+psd+eps)
  out = real(ifft2(Z*Wf))
with ri, psd derived from the 25x25 PSF on the host (tiny).

2D FFT via digit decomposition h = 8a+j, w = 8b+m, k_h = (2q+t)+128*kj,
k_w = kb+128*km. Pipeline (planes A/B alternate, all [128,8192] bf16):
  stageA: fused S1+T1 - y-blocks as stationary (lhsT), W1_j moving
          -> B[p=b, f=(m,j,k1)]  (contracts a, transposes b to partitions)
  S2:     per m contract b with F128*tw_m        -> A[p=kb, f=interleave]
  T2:     PE transpose                           -> B[p=(m,j,t), f=(q,kb)]
  S3:     contract (j,m) with D8xD8              -> zr/zi chunks
  Wiener: elementwise (chunked, 3-engine balanced)
  S3'+T2' fused: zw-blocks as stationary, conj(W3) moving
          -> A[p=kb, f=(q,g2)]   (contracts spec, transposes kb up)
  S2':    per m contract kb with conj(F128)*exp(2pi i m kb/N) (twiddle
          folded into 8 per-m matrices)          -> B[p=b, f=(q,t,j,m)]
  T1':    PE transpose                           -> A[p=k1, f=(j,m,b)]
  S1':    per j contract k1 (real out)           -> out[p=a, f=(j,8b+m)]
All spectral coefficient planes permuted on host into device layout.
Elementwise work (PSUM evacs + Wiener) is spread across DVE/ACT/Pool by a
static greedy balancer; ACT uses the reciprocal_and_small table (Square,
Reciprocal, Copy - no table reloads).
"""
import sys

sys.path.insert(0, "/opt/trn_rl_repo")

import numpy as np
import ml_dtypes

import concourse.bass as bass
import concourse.bacc as bacc
import concourse.tile as tile
from concourse import mybir
from concourse.bass_utils import run_bass_kernel_spmd
import concourse.hw_specs as _hw_specs

_orig_get_tables = _hw_specs.get_activation_tables


def _patched_tables(arch):
    t = dict(_orig_get_tables(arch))
    pin = {
        mybir.ActivationFunctionType.Ln,
        mybir.ActivationFunctionType.Exp,
        mybir.ActivationFunctionType.Square,
    }
    for name in list(t):
        if name != "natural_log_exp_and_others" and (t[name] & pin):
            t[name] = t[name] - pin
    return t


bacc.get_activation_tables = _patched_tables

N = 1024
SIGMA = 0.05
CSUB = float(SIGMA**2 * N * N * N * N)  # psd = sigma^2 * n^2 * a
N_CORES = 8
IMGS = 3  # images per core
CH = 1024  # wiener chunk cols

BDT = mybir.dt.bfloat16
F32 = mybir.dt.float32
BF = ml_dtypes.bfloat16
AF = mybir.ActivationFunctionType
ALU = mybir.AluOpType


# ---------------------------------------------------------------- host math
def _host_consts(psf25: np.ndarray) -> dict[str, np.ndarray]:
    ar = np.arange(128)
    F128 = np.exp(-2j * np.pi * np.outer(ar, ar) / 128)
    D8 = np.exp(-2j * np.pi * np.outer(np.arange(8), np.arange(8)) / 8)
    tw = np.exp(-2j * np.pi * np.outer(np.arange(8), ar) / N)  # [j, k1]

    # forward W1 per j (moving operand of fused stage-A): [a, (j, c2, k1)]
    w1f = np.empty((128, 8, 2, 128), np.float32)
    for j in range(8):
        Wj = F128 * tw[j][None, :]
        w1f[:, j, 0] = Wj.real
        w1f[:, j, 1] = Wj.imag
    # w2f: same matrices, planes (re, im, -im)
    w2f = np.empty((128, 8, 3, 128), np.float32)
    for m in range(8):
        Wm = F128 * tw[m][None, :]
        w2f[:, m, 0] = Wm.real
        w2f[:, m, 1] = Wm.imag
        w2f[:, m, 2] = -Wm.imag
    # W3 fwd: rows g_in=16m+2j+t (T2 col enum), cols spec p=16kj+2km+t
    # W3i inv: rows spec p, cols g2=64t+8j+m
    W3 = np.zeros((128, 128), complex)
    W3i = np.zeros((128, 128), complex)
    for j in range(8):
        for m in range(8):
            for kj in range(8):
                for km in range(8):
                    v = D8[j, kj] * D8[m, km]
                    for t in range(2):
                        W3[16 * m + 2 * j + t, 16 * kj + 2 * km + t] = v
                        W3i[16 * kj + 2 * km + t, 64 * t + 8 * j + m] = np.conj(v)
    w3f = np.stack([W3.real, W3.imag, -W3.imag], 1).astype(np.float32)  # [128,3,128]
    w3i = np.stack([W3i.real, W3i.imag, -W3i.imag], 1).astype(np.float32)
    # inverse S2' lhsT per m: [kb, b] = conj(F128) * exp(+2pi i m kb / N)
    # (inverse W-axis twiddle folded in); planes (re, im, -im)
    wfim = np.empty((128, 8, 3, 128), np.float32)
    kb = np.arange(128)
    for m in range(8):
        Vm = np.conj(F128) * np.exp(2j * np.pi * m * kb / N)[:, None]
        wfim[:, m, 0] = Vm.real
        wfim[:, m, 1] = Vm.imag
        wfim[:, m, 2] = -Vm.imag
    # inverse S1' lhsT per j: [k1, a] = conj(W1_j).T ; planes (re, -im) (real out)
    w1i = np.empty((128, 8, 2, 128), np.float32)
    for j in range(8):
        V = np.conj(F128 * tw[j][None, :]).T
        w1i[:, j, 0] = V.real
        w1i[:, j, 1] = -V.imag
    # wiener planes in spectral device layout
    P = np.zeros((N, N))
    P[:25, :25] = psf25
    P = np.roll(P, (-12, -12), axis=(0, 1))
    Hf = np.fft.fft2(P)
    ri = np.conj(Hf) / (np.abs(Hf) ** 2 + SIGMA**2)
    p = np.arange(128)
    kj, km, t = p // 16, (p % 16) // 2, p % 2
    f = np.arange(8192)
    q, kbf = f // 128, f % 128
    kh = (2 * q[None, :] + t[:, None]) + 128 * kj[:, None]
    kw = kbf[None, :] + 128 * km[:, None]
    rr_dev = (ri.real / (N * N))[kh, kw]
    rii_dev = (ri.imag / (N * N))[kh, kw]
    wien = np.stack([rr_dev, rii_dev], 1).astype(np.float32)  # [128,2,8192]

    bf = lambda x: np.ascontiguousarray(x.astype(BF))
    return {
        "w1f": bf(w1f.reshape(128, 8 * 2 * 128)),
        "w2f": bf(w2f.reshape(128, 8 * 3 * 128)),
        "w3f": bf(w3f.reshape(128, 3 * 128)),
        "w3i": bf(w3i.reshape(128, 3 * 128)),
        "wfim": bf(wfim.reshape(128, 8 * 3 * 128)),
        "w1i": bf(w1i.reshape(128, 8 * 2 * 128)),
        "wien": bf(wien.reshape(128, 2 * 8192)),
        "ident": bf(np.eye(128, dtype=np.float32)),
    }


# ---------------------------------------------------------------- balancer
class EW:
    """Static greedy balancer for elementwise work across DVE/ACT/Pool."""

    def __init__(self, nc, pool_psum=False):
        self.nc = nc
        self.pool_psum = pool_psum
        self.load = {"v": 0.0, "a": 0.0, "p": 0.0}

    def _cost(self, e, free, two_byte, psum, kind="tt"):
        if e == "v":
            return free * 1.04 * (0.5 if two_byte else 1.0) + (130 if psum else 62) + 70
        if e == "a":
            return free * 0.833 + 185 + 57
        # Pool (Q7 software): copy at 0.6 eff, mult/add at 0.42; tensor_scalar
        # is catastrophically slow (~15us) - never placed here.
        eff = 0.6 if kind == "copy" else 0.42
        return free * 0.833 / eff + 131

    def _pick(self, allowed, free, two_byte, psum, kind="tt"):
        e = min(
            allowed,
            key=lambda x: self.load[x] + self._cost(x, free, two_byte, psum, kind),
        )
        self.load[e] += self._cost(e, free, two_byte, psum, kind)
        return e

    def evac(self, dst, src):
        free = int(np.prod(src.shape[1:]))
        two = mybir.dt.size(src.dtype) == 2 and mybir.dt.size(dst.dtype) == 2
        allowed = ["v", "a"] + (["p"] if self.pool_psum else [])
        e = self._pick(allowed, free, two, True, "copy")
        if e == "v":
            self.nc.vector.tensor_copy(dst, src)
        elif e == "a":
            self.nc.scalar.copy(dst, src)
        else:
            self.nc.gpsimd.tensor_copy(dst, src)

    def tt(self, op, dst, a, b):
        free = int(np.prod(dst.shape[1:]))
        e = self._pick(["v", "p"], free, True, False)
        eng = self.nc.vector if e == "v" else self.nc.gpsimd
        eng.tensor_tensor(dst, a, b, op)

    def sq(self, dst, src):
        # square: ACT Square or mul on DVE/Pool
        free = int(np.prod(dst.shape[1:]))
        e = self._pick(["v", "a", "p"], free, True, False)
        if e == "a":
            self.nc.scalar.activation(dst, src, AF.Square)
        else:
            eng = self.nc.vector if e == "v" else self.nc.gpsimd
            eng.tensor_tensor(dst, src, src, ALU.mult)

    def ts(self, dst, src, s0, s1, op0, op1):
        free = int(np.prod(dst.shape[1:]))
        self.load["v"] += self._cost("v", free, True, False)
        self.nc.vector.tensor_scalar(dst, src, s0, s1, op0=op0, op1=op1)

    def maxs(self, dst, src, s0):
        free = int(np.prod(dst.shape[1:]))
        self.load["v"] += self._cost("v", free, True, False)
        self.nc.vector.tensor_scalar_max(dst, src, s0)

    def act(self, dst, src, func, scale=1.0):
        free = int(np.prod(dst.shape[1:]))
        self.load["a"] += self._cost("a", free, True, False)
        self.nc.scalar.activation(dst, src, func, scale=scale)


# ---------------------------------------------------------------- device IR
def build_program(n_imgs: int = IMGS, dbg_stage: str | None = None):
    nc = bacc.Bacc("TRN2", target_bir_lowering=False, debug=False)
    y3 = nc.dram_tensor("y3", [n_imgs, N, N], F32, kind="ExternalInput")
    o3 = nc.dram_tensor("o3", [n_imgs, N, N], F32, kind="ExternalOutput")
    if dbg_stage:
        dbr = nc.dram_tensor("dbgr", [128, 8192], F32, kind="ExternalOutput")
        dbi = nc.dram_tensor("dbgi", [128, 8192], F32, kind="ExternalOutput")
    dw = {
        k: nc.dram_tensor(k, list(s), BDT, kind="ExternalInput")
        for k, s in {
            "w1f": (128, 2048),
            "w2f": (128, 3072),
            "w3f": (128, 384),
            "w3i": (128, 384),
            "wfim": (128, 3072),
            "w1i": (128, 2048),
            "wien": (128, 16384),
            "ident": (128, 128),
        }.items()
    }

    with tile.TileContext(nc) as tc:
        import contextlib

        with contextlib.ExitStack() as ctx:
            const = ctx.enter_context(tc.tile_pool(name="const", bufs=1))
            plan = ctx.enter_context(tc.tile_pool(name="plan", bufs=1))
            ypool = ctx.enter_context(tc.tile_pool(name="ypool", bufs=2))
            ps = ctx.enter_context(tc.tile_pool(name="ps", bufs=4, space="PSUM"))
            tmp = ctx.enter_context(tc.tile_pool(name="tmp", bufs=8))
            zw = ctx.enter_context(tc.tile_pool(name="zw", bufs=5))

            ew = EW(nc, pool_psum=False)

            # constants
            sw = {}
            for k in dw:
                shp = [128, dw[k].shape[1]]
                t_ = const.tile(shp, BDT, name=k)
                nc.sync.dma_start(out=t_, in_=dw[k].ap())
                sw[k] = t_
            w1fv = sw["w1f"].rearrange("p (j ck) -> p j ck", j=8)  # ck = (c2,k1)
            w2f = sw["w2f"].rearrange("p (m c k) -> p m c k", m=8, c=3)
            w3f = sw["w3f"].rearrange("p (c k) -> p c k", c=3)
            w3iv = sw["w3i"].rearrange("p (c k) -> p c k", c=3)
            wfim = sw["wfim"].rearrange("p (m c k) -> p m c k", m=8, c=3)
            w1i = sw["w1i"].rearrange("p (j c k) -> p j c k", j=8, c=2)
            wien = sw["wien"].rearrange("p (c f) -> p c f", c=2)
            ident = sw["ident"]

            def _snap(stage, br, bi):
                if dbg_stage == stage:
                    nc.gpsimd.dma_start(out=dbr.ap(), in_=br)
                    nc.gpsimd.dma_start(out=dbi.ap(), in_=bi)

            # persistent plan buffers (bf16 [128, 8192] each; B planes merged
            # so stage-A can evacuate re+im in one instruction)
            Ari = plan.tile([128, 16384], BDT, name="Ari")
            Ar = Ari[:, :8192]
            Ai = Ari[:, 8192:]
            Bri = plan.tile([128, 16384], BDT, name="Bri")
            Br = Bri[:, :8192]
            Bi = Bri[:, 8192:]

            for img in range(n_imgs):
                # ---- load (fp32 HBM -> bf16 SBUF, SWDGE cast)
                y_t = ypool.tile([128, 8192], BDT)
                nc.gpsimd.dma_start(
                    out=y_t, in_=y3.ap()[img].rearrange("(p j) w -> p (j w)", j=8)
                )

                # ---- stage A (fused S1+T1): y blocks stationary, W1_j moving
                # out B[p=b, f = 1024m + 128j + k1] (complex)
                yv = y_t.rearrange("p (j b m) -> p j m b", j=8, b=128)
                for m in range(8):
                    for g in range(2):  # j groups of 4 -> one [128,1024] psum tile
                        pt = ps.tile([128, 1024], F32, tag="pp")
                        for jj in range(4):
                            j = 4 * g + jj
                            nc.tensor.matmul(
                                pt[:, 256 * jj : 256 * (jj + 1)],
                                yv[:, j, m],
                                w1fv[:, j],
                                start=True,
                                stop=True,
                            )
                        ptv = pt.rearrange("p (jj c k) -> p c jj k", jj=4, c=2)
                        dst = Bri.rearrange(
                            "p (pl mm gg jj k) -> p mm gg pl jj k",
                            pl=2, mm=8, gg=2, jj=4,
                        )[:, m, g]
                        ew.evac(dst, ptv)

                _snap("A", Br, Bi)
                # ---- S2: per m contract b -> A[p=kb, f=interleaved]
                for m in range(8):
                    for c in range(2):
                        jr = Br[:, 1024 * m + 512 * c : 1024 * m + 512 * (c + 1)]
                        ji = Bi[:, 1024 * m + 512 * c : 1024 * m + 512 * (c + 1)]
                        prc = ps.tile([128, 1024], F32, tag="pp")
                        pr = prc[:, :512]
                        pi = prc[:, 512:]
                        nc.tensor.matmul(pr, w2f[:, m, 0], jr, start=True, stop=False)
                        nc.tensor.matmul(pi, w2f[:, m, 1], jr, start=True, stop=False)
                        nc.tensor.matmul(pr, w2f[:, m, 2], ji, start=False, stop=True)
                        nc.tensor.matmul(pi, w2f[:, m, 0], ji, start=False, stop=True)
                        # interleaved evac: psum (jj,q,t) -> f = 128q + 16m + 8c + 2jj + t
                        for dstp, srcp in ((Ar, pr), (Ai, pi)):
                            sview = srcp.rearrange("p (jj q t) -> p jj q t", jj=4, q=64)
                            dview = dstp.rearrange(
                                "p (q mm cc jj t) -> p q mm cc jj t",
                                q=64, mm=8, cc=2, jj=4,
                            )[:, :, m, c, :, :].transpose([0, 2, 1, 3])
                            ew.evac(dview, sview)

                _snap("S2", Ar, Ai)
                # ---- T2: A -> B[p=g_in=(m,j,t), f=(q,kb)]
                for g in range(8):  # groups of 8 q
                    for srcv, dst in ((Ar, Br), (Ai, Bi)):
                        pt = ps.tile([128, 1024], BDT, tag="pp")
                        for qq in range(8):
                            q = 8 * g + qq
                            nc.tensor.transpose(
                                pt[:, 128 * qq : 128 * (qq + 1)],
                                srcv[:, 128 * q : 128 * (q + 1)],
                                ident,
                            )
                        ew.evac(dst[:, 1024 * g : 1024 * (g + 1)], pt)

                _snap("T2", Br, Bi)
                # ---- S3 + Wiener + fused S3'+T2' : B -> A[p=kb, f=(q,g2)]
                # S3'+T2' matmuls are deferred PRE chunks so the in-order PE
                # queue never waits on a chunk's Wiener chain.
                nch = 8192 // CH
                PRE = 3
                zwris = {}
                for cc_ in range(nch + PRE):
                    if cc_ < nch:
                        c = cc_
                        sl = slice(CH * c, CH * (c + 1))
                        zri = zw.tile([128, 2 * CH], BDT, tag="zri", bufs=2)
                        zr = zri[:, :CH]
                        zi = zri[:, CH:]
                        for hh in range(CH // 512):
                            bsl = slice(CH * c + 512 * hh, CH * c + 512 * (hh + 1))
                            pt = ps.tile([128, 1024], F32, tag="pp")
                            pr = pt[:, :512]
                            pi = pt[:, 512:]
                            nc.tensor.matmul(pr, w3f[:, 0], Br[:, bsl], start=True, stop=False)
                            nc.tensor.matmul(pi, w3f[:, 1], Br[:, bsl], start=True, stop=False)
                            nc.tensor.matmul(pr, w3f[:, 2], Bi[:, bsl], start=False, stop=True)
                            nc.tensor.matmul(pi, w3f[:, 0], Bi[:, bsl], start=False, stop=True)
                            dz = zri.rearrange("p (pl h k) -> p h pl k", pl=2, h=CH // 512)[:, hh]
                            ew.evac(dz, pt.rearrange("p (pl k) -> p pl k", pl=2))
                        rrc = wien[:, 0, sl]
                        ric = wien[:, 1, sl]
                        # shallow Wiener: e1/e2 built in parallel with the w chain
                        t1 = tmp.tile([128, CH], BDT, tag="wt")
                        nc.scalar.activation(t1, zr, AF.Square)
                        t2 = tmp.tile([128, CH], BDT, tag="wt")
                        nc.scalar.activation(t2, zi, AF.Square)
                        mm_ = tmp.tile([128, CH], BDT, tag="wt")
                        nc.vector.tensor_tensor(mm_, t1, t2, ALU.add)
                        rc = tmp.tile([128, CH], BDT, tag="wt")
                        nc.vector.tensor_scalar(rc, mm_, CSUB, 0.0, op0=ALU.subtract, op1=ALU.max)
                        u2 = tmp.tile([128, CH], F32, tag="wtf", bufs=2)
                        nc.vector.tensor_scalar_max(u2, mm_, CSUB)
                        ln_ = tmp.tile([128, CH], F32, tag="wtf", bufs=2)
                        nc.scalar.activation(ln_, u2, AF.Ln)
                        r_ = tmp.tile([128, CH], BDT, tag="wt")
                        nc.scalar.activation(r_, ln_, AF.Exp, scale=-1.0)
                        w_ = tmp.tile([128, CH], BDT, tag="wt")
                        nc.vector.tensor_tensor(w_, rc, r_, ALU.mult)
                        zrr = tmp.tile([128, CH], BDT, tag="wt")
                        nc.vector.tensor_tensor(zrr, zr, rrc, ALU.mult)
                        zii = tmp.tile([128, CH], BDT, tag="wt")
                        nc.gpsimd.tensor_tensor(zii, zi, ric, ALU.mult)
                        zri2 = tmp.tile([128, CH], BDT, tag="wt")
                        nc.vector.tensor_tensor(zri2, zr, ric, ALU.mult)
                        zir = tmp.tile([128, CH], BDT, tag="wt")
                        nc.gpsimd.tensor_tensor(zir, zi, rrc, ALU.mult)
                        e1 = tmp.tile([128, CH], BDT, tag="wt")
                        nc.vector.tensor_tensor(e1, zrr, zii, ALU.subtract)
                        e2 = tmp.tile([128, CH], BDT, tag="wt")
                        nc.vector.tensor_tensor(e2, zri2, zir, ALU.add)
                        zwri = zw.tile([128, 2 * CH], BDT, tag="zwri")
                        nc.vector.tensor_tensor(zwri[:, :CH], e1, w_, ALU.mult)
                        nc.vector.tensor_tensor(zwri[:, CH:], e2, w_, ALU.mult)
                        zwris[c] = zwri
                    if cc_ >= PRE:
                        c = cc_ - PRE
                        sl = slice(CH * c, CH * (c + 1))
                        zwri = zwris.pop(c)
                        zwr = zwri[:, :CH]
                        zwi = zwri[:, CH:]
                        ptr = ps.tile([128, CH], F32, tag="pp")
                        pti = ps.tile([128, CH], F32, tag="pp")
                        for qq in range(CH // 128):
                            qsl = slice(128 * qq, 128 * (qq + 1))
                            nc.tensor.matmul(ptr[:, qsl], zwr[:, qsl], w3iv[:, 0], start=True, stop=False)
                            nc.tensor.matmul(pti[:, qsl], zwr[:, qsl], w3iv[:, 1], start=True, stop=False)
                            nc.tensor.matmul(ptr[:, qsl], zwi[:, qsl], w3iv[:, 2], start=False, stop=True)
                            nc.tensor.matmul(pti[:, qsl], zwi[:, qsl], w3iv[:, 0], start=False, stop=True)
                        ew.evac(Ar[:, sl], ptr)
                        ew.evac(Ai[:, sl], pti)

                _snap("S3p", Ar, Ai)
                # ---- S2': per m contract kb (twiddle-folded conj(F128)) -> B[p=b, f=(q,t,j,m)]
                Avr = Ar.rearrange("p (q t j m) -> p q t j m", q=64, t=2, j=8)
                Avi = Ai.rearrange("p (q t j m) -> p q t j m", q=64, t=2, j=8)
                for m in range(8):
                    for c in range(2):
                        qsl = slice(32 * c, 32 * (c + 1))
                        jr = Avr[:, qsl, :, :, m]
                        ji = Avi[:, qsl, :, :, m]
                        prc = ps.tile([128, 1024], F32, tag="pp")
                        pr = prc[:, :512]
                        pi = prc[:, 512:]
                        nc.tensor.matmul(pr, wfim[:, m, 0], jr, start=True, stop=False)
                        nc.tensor.matmul(pi, wfim[:, m, 1], jr, start=True, stop=False)
                        nc.tensor.matmul(pr, wfim[:, m, 2], ji, start=False, stop=True)
                        nc.tensor.matmul(pi, wfim[:, m, 0], ji, start=False, stop=True)
                        dst = Bri.rearrange(
                            "p (pl mm cc k) -> p mm cc pl k", pl=2, mm=8, cc=2
                        )[:, m, c]
                        ew.evac(dst, prc.rearrange("p (pl k) -> p pl k", pl=2))

                _snap("S2p", Br, Bi)
                # ---- T1': B[p=b, f=(m,c,q32,t,j)] -> A[p=k1, f=(j,m,b)]
                vB4r = Br.rearrange(
                    "p (mm cc q32 t j) -> p j mm cc q32 t", mm=8, cc=2, q32=32, t=2
                )
                vB4i = Bi.rearrange(
                    "p (mm cc q32 t j) -> p j mm cc q32 t", mm=8, cc=2, q32=32, t=2
                )
                for j in range(8):
                    for srcv, dst in ((vB4r, Ar), (vB4i, Ai)):
                        pt = ps.tile([128, 1024], BDT, tag="pp")
                        for mj in range(8):
                            nc.tensor.transpose(
                                pt[:, 128 * mj : 128 * (mj + 1)], srcv[:, j, mj], ident
                            )
                        ew.evac(dst[:, 1024 * j : 1024 * (j + 1)], pt)

                _snap("T1p", Ar, Ai)
                # ---- S1': per j contract k1 (real out) -> out[p=a, f=(j, 8b+m)]
                out_t = ypool.tile([128, 8192], BDT, tag="y_t")
                for j in range(8):
                    for c in range(2):
                        off = 1024 * j + 512 * c
                        jr = Ar[:, off : off + 512]
                        ji = Ai[:, off : off + 512]
                        pr = ps.tile([128, 512], F32, tag="pp")
                        nc.tensor.matmul(pr, w1i[:, j, 0], jr, start=True, stop=False)
                        nc.tensor.matmul(pr, w1i[:, j, 1], ji, start=False, stop=True)
                        # evac with digit swap (m,b) -> 8b+m
                        src = pr.rearrange("p (m b) -> p m b", m=4)
                        dst = out_t.rearrange("p (j b m) -> p j b m", j=8, b=128)[
                            :, j, :, 4 * c : 4 * (c + 1)
                        ].transpose([0, 2, 1])
                        ew.evac(dst, src)
                _snap("OUT", out_t, out_t)
                nc.gpsimd.dma_start(
                    out=o3.ap()[img].rearrange("(p j) w -> p (j w)", j=8), in_=out_t
                )

    nc.compile()
    return nc


_PROG = None


def _get_prog():
    global _PROG
    if _PROG is None:
        _PROG = build_program(IMGS)
    return _PROG


def kernel(y: np.ndarray, psf: np.ndarray) -> np.ndarray:
    consts = _host_consts(np.asarray(psf, np.float64)[0, 0])
    nc = _get_prog()
    y24 = np.ascontiguousarray(np.asarray(y, np.float32).reshape(N_CORES * IMGS, N, N))
    in_maps = []
    for c in range(N_CORES):
        m = dict(consts)
        m["y3"] = y24[IMGS * c : IMGS * (c + 1)]
        in_maps.append(m)
    res = run_bass_kernel_spmd(nc, in_maps, core_ids=list(range(N_CORES)))
    out = np.stack([res.results[c]["o3"] for c in range(N_CORES)])
    return out.reshape(8, 3, N, N).astype(np.float32)
